# revision 1
# baseline (speedup 1.0000x reference)
# Bass/Trainium2 kernel for nn_Delta (DeltaNet-style recurrence).
#
# Problem (hardcoded): N=8, T=2048, C=512, fp32 I/O.
#   g = x @ Wg.T + bg ; q,k,v = split(g) ; lr = x @ Wlr.T + blr
#   k = k / ||k||
#   delta-rule scan:  u_t = v_t - k_t @ S ; S += outer(k_t, u_t)
#   kv = sum_t k_t (x) (lr_t * u_t) ; y = q @ kv ; out = y @ Wo.T + bo
#
# Sharding: data-parallel over N across the 8 cores (sample i -> core i),
# weights replicated. No collectives.
#
# Per-core algorithm: chunked parallel delta rule with chunk L=128.
#   Within a chunk: (I + A) U = V - K @ S0, A = tril(K K^T, -1).
#   (I+A)^-1 = (I+F1)(I+F2)(I+F3)(I+F4) with commuting factors built from
#   powers of B = -A (base-4 digits):
#     F1 = B+B^2+B^3, F2 = B^4+B^8+B^12, F3 = B^16+B^32+B^48, F4 = B^64.
#   Powers are maintained as (upper, lower) transpose pairs so every matmul
#   has its lhsT available without explicit transposes.
#
# Output path uses associativity to avoid materializing q:
#   y Wo^T = q @ kv @ Wo^T = x @ (Wgq^T kv Wo^T) + bgq @ (kv Wo^T)
# so the T-sized projections are only x->K, x->V, x->out.
#
# Layout strategy: a few large cast-DMA loads (f32->bf16), all transposes
# done on the TensorEngine via identity matmuls (no DMA transposes). The
# power-chain PSUM pools are declared first so they land on banks released
# early, letting each chunk's (state-independent) inverse factors compute
# during the projection phase; the serial solve then streams with the
# factors already in SBUF.

import numpy as np

N, T, C = 8, 2048, 512
L = 128
NCH = T // L  # 16 chunks
CT = C // 128  # 4 c-tiles

_CACHE = {}


def _build():
    import concourse.bacc as bacc
    import concourse.mybir as mybir
    import concourse.tile as tile
    from concourse.bass import ts, ds
    from concourse.masks import (
        make_identity,
        make_lower_triangular,
        make_upper_triangular,
    )

    f32 = mybir.dt.float32
    f32r = mybir.dt.float32r
    bf16 = mybir.dt.bfloat16
    AF = mybir.ActivationFunctionType

    nc = bacc.Bacc("TRN2")
    x = nc.declare_dram_parameter("x", [T, C], f32, isOutput=False)
    Wg = nc.declare_dram_parameter("Wg", [3 * C, C], f32, isOutput=False)
    bg = nc.declare_dram_parameter("bg", [3 * C], f32, isOutput=False)
    Wlr = nc.declare_dram_parameter("Wlr", [1, C], f32, isOutput=False)
    blr = nc.declare_dram_parameter("blr", [1], f32, isOutput=False)
    Wo = nc.declare_dram_parameter("Wo", [C, C], f32, isOutput=False)
    bo = nc.declare_dram_parameter("bo", [C], f32, isOutput=False)
    out = nc.declare_dram_parameter("out", [T, C], f32, isOutput=True)

    mm = nc.tensor.matmul

    with tile.TileContext(nc) as tc:
        with tc.tile_pool(name="persist", bufs=1) as P:
            # ---- constants / small tensors ----
            maskU = P.tile([128, 128], f32, name="maskU")
            maskL = P.tile([128, 128], f32, name="maskL")
            ones_bf = P.tile([1, 128], bf16, name="ones_bf")
            nc.vector.memset(ones_bf[:], 1.0)
            ident_bf = P.tile([128, 128], bf16, name="ident_bf")
            maskU2 = P.tile([128, 256], f32, name="maskU2")
            maskL2 = P.tile([128, 256], f32, name="maskL2")
            ident_f32 = P.tile([128, 128], f32, name="ident_f32")
            ident_fr = P.tile([128, 128], f32r, name="ident_fr")

            # ---- persistent tensors ----
            xT = [P.tile([128, T], bf16, name=f"xT{i}") for i in range(CT)]
            WgTk = [P.tile([128, C], bf16, name=f"WgTk{i}") for i in range(CT)]
            WgTv = [P.tile([128, C], bf16, name=f"WgTv{i}") for i in range(CT)]
            WoT = [P.tile([128, C], bf16, name=f"WoT{i}") for i in range(CT)]
            wgq_sb = P.tile([128, 4 * C], bf16, name="wgq_sb")
            Khat = [P.tile([128, C], f32r, name=f"Khat{i}") for i in range(NCH)]
            # chunk-major K^T: KTc[i][:, ci*128:(ci+1)*128] = Khat_i^T c-block
            KTc = [P.tile([128, C], bf16, name=f"KTc{i}") for i in range(NCH)]
            U = [P.tile([128, C], f32r, name=f"U{i}") for i in range(NCH)]
            # Sn = -S (negated state, bf16 accumulation) so the K@S term
            # accumulates positively in the solve
            S_bf = [P.tile([128, C], bf16, name=f"Sb{i}") for i in range(CT)]
            KVT = [P.tile([128, C], bf16, name=f"KVT{i}") for i in range(CT)]
            KVW = [P.tile([128, C], bf16, name=f"KVW{i}") for i in range(CT)]
            Mw = [P.tile([128, C], bf16, name=f"Mw{i}") for i in range(CT)]
            rb = P.tile([1, C], bf16, name="rb")
            lrn = P.tile([128, NCH], f32, name="lrn")
            KVf = [P.tile([128, C], f32, name=f"KVf{i}") for i in range(CT)]

            # ============ phase A: loads + PE transposes + K projections ====
            with tc.tile_pool(name="stage", bufs=1) as STG, \
                 tc.tile_pool(name="sbB", bufs=3) as SBB, \
                 tc.tile_pool(name="small", bufs=6) as SMALL, \
                 tc.tile_pool(name="psTR", bufs=2, space="PSUM") as TRP, \
                 tc.tile_pool(name="psTK", bufs=1, space="PSUM") as TRK, \
                 tc.tile_pool(name="psB", bufs=2, space="PSUM") as PSB, \
                 tc.tile_pool(name="psLr", bufs=1, space="PSUM") as PSLR:
                # large cast loads, rearranged so row-chunk j lands in
                # column block j: sb[p, j*512 + c] = src[j*128 + p, c]
                def big_load(dst, src_rows):
                    nc.gpsimd.dma_start(
                        out=dst[:].rearrange("p (j c) -> p j c", j=4),
                        in_=src_rows.rearrange("(j p) c -> p j c", p=128),
                    )

                x_sb = [STG.tile([128, 4 * C], bf16, name=f"x_sb{g}")
                        for g in range(4)]
                wgk_sb = STG.tile([128, 4 * C], bf16, name="wgk_sb")
                big_load(wgk_sb, Wg[C:2 * C, :])
                make_identity(nc, ident_bf[:])
                big_load(x_sb[0], x[0:512, :])
                wgv_sb = STG.tile([128, 4 * C], bf16, name="wgv_sb")
                big_load(wgv_sb, Wg[2 * C:3 * C, :])
                for g in range(1, 4):
                    big_load(x_sb[g], x[g * 512:(g + 1) * 512, :])
                big_load(wgq_sb, Wg[0:C, :])
                wo_sb = STG.tile([128, 4 * C], bf16, name="wo_sb")
                big_load(wo_sb, Wo[:, :])

                bgk_sb = P.tile([1, C], bf16, name="bgk_sb")
                nc.gpsimd.dma_start(out=bgk_sb[:], in_=bg[C:2 * C])
                bgv_sb = P.tile([1, C], bf16, name="bgv_sb")
                nc.gpsimd.dma_start(out=bgv_sb[:], in_=bg[2 * C:3 * C])
                bo_sb = P.tile([1, C], bf16, name="bo_sb")
                nc.gpsimd.dma_start(out=bo_sb[:], in_=bo[:])
                bgqT = P.tile([128, CT], bf16, name="bgqT")
                nc.gpsimd.dma_start(
                    out=bgqT[:], in_=bg[0:C].rearrange("(i p) -> p i", p=128)
                )
                WlrT = P.tile([128, CT], bf16, name="WlrT")
                nc.gpsimd.dma_start(
                    out=WlrT[:], in_=Wlr[0, :].rearrange("(i p) -> p i", p=128)
                )
                blr_sb = P.tile([1, 1], bf16, name="blr_sb")
                nc.gpsimd.dma_start(out=blr_sb[:], in_=blr[:])

                # masks / remaining identities (needed from ~25us on)
                make_upper_triangular(nc, maskU[:], val=-1.0, diag=False)
                make_lower_triangular(nc, maskL[:], val=-1.0, diag=False)
                nc.vector.tensor_copy(maskU2[:, 0:128], maskU[:])
                nc.vector.tensor_copy(maskU2[:, 128:256], maskU[:])
                nc.vector.tensor_copy(maskL2[:, 0:128], maskL[:])
                nc.vector.tensor_copy(maskL2[:, 128:256], maskL[:])
                make_identity(nc, ident_f32[:])
                nc.vector.tensor_copy(ident_fr[:], ident_f32[:])

                # PE transposes: [128,128] tiles via identity matmul.
                def transpose_tiles(src, ci):
                    tp = TRP.tile([128, 4 * 128], bf16, name="tp")
                    for j in range(4):
                        nc.tensor.transpose(
                            tp[:, ts(j, 128)],
                            src[:, ds(j * 512 + ci * 128, 128)],
                            ident_bf[:],
                        )
                    return tp

                for ci in range(CT):
                    tp = transpose_tiles(wgk_sb, ci)
                    nc.any.tensor_copy(WgTk[ci][:], tp[:])
                for ci in range(CT):
                    tp = transpose_tiles(x_sb[0], ci)
                    nc.any.tensor_copy(xT[ci][:, ds(0, 512)], tp[:])
                for ci in range(CT):
                    tp = transpose_tiles(wgv_sb, ci)
                    nc.any.tensor_copy(WgTv[ci][:], tp[:])
                for g in range(1, 4):
                    for ci in range(CT):
                        tp = transpose_tiles(x_sb[g], ci)
                        nc.any.tensor_copy(xT[ci][:, ds(g * 512, 512)], tp[:])
                for ci in range(CT):
                    tp = transpose_tiles(wo_sb, ci)
                    nc.any.tensor_copy(WoT[ci][:], tp[:])

                # K projection + normalization; K^T per chunk (bf16)
                for tj in range(NCH):
                    kps = PSB.tile([128, C], f32, name="kps")
                    for ci in range(CT):
                        mm(kps[:], lhsT=xT[ci][:, ts(tj, 128)], rhs=WgTk[ci][:],
                           start=(ci == 0), stop=False)
                    mm(kps[:], lhsT=ones_bf[:], rhs=bgk_sb[:], start=False,
                       stop=True)
                    sq = SBB.tile([128, C], f32, name="sq")
                    n2 = SMALL.tile([128, 1], f32, name="n2")
                    nc.scalar.activation(sq[:], kps[:], AF.Square, accum_out=n2[:])
                    nrm = SMALL.tile([128, 1], f32, name="nrm")
                    nc.scalar.sqrt(nrm[:], n2[:])
                    rn = SMALL.tile([128, 1], f32, name="rn")
                    nc.vector.reciprocal(rn[:], nrm[:])
                    nc.vector.tensor_scalar_mul(Khat[tj][:], kps[:], rn[:])
                    ktp = TRK.tile([128, C], f32r, name="ktp")
                    for ci in range(CT):
                        nc.tensor.transpose(
                            ktp[:, ts(ci, 128)],
                            Khat[tj][:, ts(ci, 128)],
                            ident_fr[:],
                        )
                    nc.any.tensor_copy(KTc[tj][:], ktp[:])

                # lr column per chunk: lrn[:, i] = x_chunk @ Wlr^T + blr
                lps = PSLR.tile([128, NCH], f32, name="lps")
                for i in range(NCH):
                    for ci in range(CT):
                        mm(lps[:, i:i + 1], lhsT=xT[ci][:, ts(i, 128)],
                           rhs=WlrT[:, ci:ci + 1], start=(ci == 0), stop=False)
                    mm(lps[:, i:i + 1], lhsT=ones_bf[:], rhs=blr_sb[:],
                       start=False, stop=True)
                nc.any.tensor_copy(lrn[:], lps[:])

            # ================= phase C: delta-rule recurrence ===============
            # psPa/psPb are declared first so they are assigned the banks
            # released earliest by phase A, letting the per-chunk inverse
            # factors (independent of the recurrence state) run ahead.
            with tc.tile_pool(name="sbP", bufs=3) as SBP, \
                 tc.tile_pool(name="sbF", bufs=3) as SBF, \
                 tc.tile_pool(name="sbM", bufs=NCH // 2) as SBM, \
                 tc.tile_pool(name="sbG", bufs=24) as SBG, \
                 tc.tile_pool(name="sbU", bufs=2) as SBU, \
                 tc.tile_pool(name="sbLR", bufs=6) as SBLR, \
                 tc.tile_pool(name="psPa", bufs=1, space="PSUM") as PSPA, \
                 tc.tile_pool(name="psPb", bufs=1, space="PSUM") as PSPB, \
                 tc.tile_pool(name="psU", bufs=4, space="PSUM") as PSU, \
                 tc.tile_pool(name="psSD", bufs=2, space="PSUM") as PSSD:
                for ci in range(CT):
                    nc.gpsimd.memset(S_bf[ci][:], 0.0)
                for vi in range(CT):
                    nc.gpsimd.memset(KVf[vi][:], 0.0)

                gxns = {}
                lrus = {}
                Mps = {}
                for i in range(NCH):
                    if i % 2 == 0:
                        # ---- lockstep inverse-factor chain for the chunk
                        # pair (a, b) = (i, i+1). Region layout of every
                        # level tile: {X_a | X_b | Xl_a | Xl_b} so each
                        # level is ONE [128,512] bank with ONE evacuation.
                        a, b = i, i + 1
                        PSP = PSPA if (i // 2) % 2 == 0 else PSPB
                        gps = PSP.tile([128, 256], f32, name="pp")
                        for ci in range(CT):
                            ka = KTc[a][:, ts(ci, 128)]
                            mm(gps[:, 0:128], lhsT=ka, rhs=ka,
                               start=(ci == 0), stop=(ci == 3))
                        for ci in range(CT):
                            kb = KTc[b][:, ts(ci, 128)]
                            mm(gps[:, 128:256], lhsT=kb, rhs=kb,
                               start=(ci == 0), stop=(ci == 3))
                        BuP = SBP.tile([128, 256], bf16, name="BuP")
                        BlP = SBP.tile([128, 256], bf16, name="BlP")
                        nc.any.tensor_mul(BuP[:], gps[:], maskU2[:])
                        nc.any.tensor_mul(BlP[:], gps[:], maskL2[:])

                        def pair2(prev_u, prev_l, name):
                            # squares both chunks: out_u = l^T u, out_l = u^T l
                            ps = PSP.tile([128, 512], f32, name="pp")
                            for s in range(2):
                                pu = prev_u[:, ts(s, 128)]
                                pl = prev_l[:, ts(s, 128)]
                                mm(ps[:, ds(s * 128, 128)], lhsT=pl, rhs=pu,
                                   start=True, stop=True)
                                mm(ps[:, ds(256 + s * 128, 128)], lhsT=pu,
                                   rhs=pl, start=True, stop=True)
                            t = SBP.tile([128, 512], bf16, name=name)
                            if (i // 2) % 2 == 0:
                                nc.vector.tensor_copy(t[:], ps[:])
                            else:
                                nc.scalar.activation(t[:], ps[:], AF.Identity)
                            return t[:, 0:256], t[:, 256:512]

                        B2uP, B2lP = pair2(BuP, BlP, "B22")
                        B4uP, B4lP = pair2(B2uP, B2lP, "B44")
                        B8uP, B8lP = pair2(B4uP, B4lP, "B88")
                        B16uP, B16lP = pair2(B8uP, B8lP, "B1616")
                        B32uP, B32lP = pair2(B16uP, B16lP, "B3232")

                        # fp1X: {B3u_a | B3u_b | B48u_a | B48u_b}
                        fp1 = PSP.tile([128, 512], f32, name="pp")
                        for s in range(2):
                            mm(fp1[:, ds(s * 128, 128)],
                               lhsT=BlP[:, ts(s, 128)], rhs=BuP[:, ts(s, 128)],
                               start=True, stop=False)
                            mm(fp1[:, ds(s * 128, 128)],
                               lhsT=B2lP[:, ts(s, 128)], rhs=BuP[:, ts(s, 128)],
                               start=False, stop=True)
                            mm(fp1[:, ds(256 + s * 128, 128)],
                               lhsT=B8lP[:, ts(s, 128)], rhs=B8uP[:, ts(s, 128)],
                               start=True, stop=False)
                            mm(fp1[:, ds(256 + s * 128, 128)],
                               lhsT=B16lP[:, ts(s, 128)], rhs=B32uP[:, ts(s, 128)],
                               start=False, stop=True)
                        F1P = SBF.tile([128, 256], bf16, name="F1P")
                        nc.any.tensor_add(F1P[:], fp1[:, 0:256], BuP[:])
                        F3P = SBF.tile([128, 256], bf16, name="F3P")
                        nc.any.tensor_add(F3P[:], fp1[:, 256:512], B32uP)

                        # fp2X: {B12u_a | B12u_b | B12l_a | B12l_b}
                        fp2 = PSP.tile([128, 512], f32, name="pp")
                        for s in range(2):
                            mm(fp2[:, ds(s * 128, 128)],
                               lhsT=B2lP[:, ts(s, 128)], rhs=B2uP[:, ts(s, 128)],
                               start=True, stop=False)
                            mm(fp2[:, ds(s * 128, 128)],
                               lhsT=B4lP[:, ts(s, 128)], rhs=B8uP[:, ts(s, 128)],
                               start=False, stop=True)
                            mm(fp2[:, ds(256 + s * 128, 128)],
                               lhsT=B2uP[:, ts(s, 128)], rhs=B2lP[:, ts(s, 128)],
                               start=True, stop=False)
                            mm(fp2[:, ds(256 + s * 128, 128)],
                               lhsT=B8uP[:, ts(s, 128)], rhs=B4lP[:, ts(s, 128)],
                               start=False, stop=True)
                        F2P = SBF.tile([128, 256], bf16, name="F2P")
                        nc.any.tensor_add(F2P[:], fp2[:, 0:256], B8uP)
                        F2lP = SBF.tile([128, 256], bf16, name="F2lP")
                        nc.any.tensor_add(F2lP[:], fp2[:, 256:512], B8lP)

                        # ps64X: {B64u_a | B64u_b | B64l_a | B64l_b}
                        ps64 = PSP.tile([128, 512], f32, name="pp")
                        for s in range(2):
                            mm(ps64[:, ds(s * 128, 128)],
                               lhsT=B32lP[:, ts(s, 128)], rhs=B32uP[:, ts(s, 128)],
                               start=True, stop=True)
                            mm(ps64[:, ds(256 + s * 128, 128)],
                               lhsT=B32uP[:, ts(s, 128)], rhs=B32lP[:, ts(s, 128)],
                               start=True, stop=True)
                        F44P = SBF.tile([128, 512], bf16, name="F44P")
                        if (i // 2) % 2 == 0:
                            nc.scalar.activation(F44P[:], ps64[:], AF.Identity)
                        else:
                            nc.vector.tensor_copy(F44P[:], ps64[:])
                        F4uP, F4lP = F44P[:, 0:256], F44P[:, 256:512]

                        # mpsX: {M12u_a | M12u_b | M34u_a | M34u_b}
                        mps = PSP.tile([128, 512], f32, name="pp")
                        for s in range(2):
                            mm(mps[:, ds(s * 128, 128)],
                               lhsT=F2lP[:, ts(s, 128)], rhs=F1P[:, ts(s, 128)],
                               start=True, stop=False)
                            mm(mps[:, ds(s * 128, 128)],
                               lhsT=ident_bf[:], rhs=F1P[:, ts(s, 128)],
                               start=False, stop=False)
                            mm(mps[:, ds(s * 128, 128)],
                               lhsT=ident_bf[:], rhs=F2P[:, ts(s, 128)],
                               start=False, stop=True)
                            mm(mps[:, ds(256 + s * 128, 128)],
                               lhsT=F4lP[:, ts(s, 128)], rhs=F3P[:, ts(s, 128)],
                               start=True, stop=False)
                            mm(mps[:, ds(256 + s * 128, 128)],
                               lhsT=ident_bf[:], rhs=F3P[:, ts(s, 128)],
                               start=False, stop=False)
                            mm(mps[:, ds(256 + s * 128, 128)],
                               lhsT=ident_bf[:], rhs=F4uP[:, ts(s, 128)],
                               start=False, stop=True)
                        MpP = SBM.tile([128, 512], bf16, name="Mp")
                        nc.any.tensor_copy(MpP[:], mps[:])
                        Mps[i // 2] = MpP

                    # cross-chunk correction matrices within the quad
                    # (independent of U/S; can run ahead):
                    # gxns[src] with gxn^T @ U_src = -K_i K_src^T U_src
                    for src in range(i - i % 2, i):
                        gx = (PSPA if i % 2 == 0 else PSPB).tile(
                            [128, 256], f32, name="pp")
                        for ci in range(CT):
                            mm(gx[:, 0:128], lhsT=KTc[src][:, ts(ci, 128)],
                               rhs=KTc[i][:, ts(ci, 128)],
                               start=(ci == 0), stop=(ci == 3))
                        g_t = SBG.tile([128, 128], f32r, name="gxn")
                        nc.vector.tensor_scalar_mul(g_t[:], gx[:, 0:128], -1.0)
                        gxns[(i, src)] = g_t

                    # --- U_i = (I+F1)(I+F2)(I+F3)(I+F4) (V_i - Khat_i S0) ---
                    # Chunks are paired: odd chunk i reads the pair-start state
                    # S0 plus an explicit cross term -A21 @ U_{i-1}.
                    ups = PSU.tile([128, C], f32, name="ups")
                    for ci in range(CT):
                        mm(ups[:], lhsT=xT[ci][:, ts(i, 128)],
                           rhs=WgTv[ci][:],
                           start=(ci == 0), stop=False)
                    mm(ups[:], lhsT=ones_bf[:], rhs=bgv_sb[:],
                       start=False, stop=(i <= 1))
                    if i >= 2:
                        for ci in range(CT):
                            mm(ups[:], lhsT=KTc[i][:, ts(ci, 128)],
                               rhs=S_bf[ci][:],
                               start=False, stop=(ci == 3))
                    for src in range(i - i % 2, i):
                        mm(ups[:], lhsT=gxns[(i, src)][:], rhs=U[src][:],
                           start=False, stop=True,
                           skip_group_check=True)
                    MpP = Mps[i // 2]
                    M12u = MpP[:, ds((i % 2) * 128, 128)]
                    M34u = MpP[:, ds(256 + (i % 2) * 128, 128)]
                    ucur = SBU.tile([128, C], bf16, name="ua")
                    nc.any.tensor_copy(ucur[:], ups[:])  # U_0 = V - Khat S0
                    mm(ups[:], lhsT=M34u, rhs=ucur[:], start=False, stop=True,
                       skip_group_check=True)
                    umid = SBU.tile([128, C], bf16, name="ub")
                    nc.any.tensor_copy(umid[:], ups[:])
                    mm(ups[:], lhsT=M12u, rhs=umid[:], start=False, stop=True,
                       skip_group_check=True)
                    nc.any.tensor_copy(U[i][:], ups[:])
                    lru = SBLR.tile([128, C], f32r, name="lru")
                    nc.vector.tensor_scalar_mul(lru[:], U[i][:], lrn[:, i:i + 1])
                    lrus[i] = lru

                    # --- Sn -= Khat^T U for the quad, at quad end (bf16) ---
                    if i % 2 == 1 and i < NCH - 1:
                        for ci in range(CT):
                            sd = PSSD.tile([128, C], f32, name="sd")
                            for j in range(2):
                                mm(sd[:], lhsT=Khat[i - 1 + j][:, ts(ci, 128)],
                                   rhs=U[i - 1 + j][:], start=(j == 0),
                                   stop=(j == 1))
                            nc.vector.tensor_sub(S_bf[ci][:], S_bf[ci][:], sd[:])
                    # --- kv partial: KVf[vi] += sum_j lru_j^T(vi) Khat_j ---
                    if i % 4 == 3:
                        last = (i == NCH - 1)
                        for vi in range(CT):
                            kvp = PSSD.tile([128, C], f32, name="sd")
                            for j in range(4):
                                cj = i - 3 + j
                                mm(kvp[:], lhsT=lrus[cj][:, ts(vi, 128)],
                                   rhs=Khat[cj][:], start=(j == 0),
                                   stop=(j == 3))
                            if last:
                                # final partial: write the bf16 KVT directly
                                nc.vector.tensor_add(KVT[vi][:], kvp[:],
                                                     KVf[vi][:])
                            else:
                                nc.vector.tensor_add(KVf[vi][:], kvp[:],
                                                     KVf[vi][:])
                        lrus.clear()

            # ================= phase D: outputs =============================
            with tc.tile_pool(name="sbD", bufs=4) as SBD, \
                 tc.tile_pool(name="sbZ", bufs=3) as SBZ, \
                 tc.tile_pool(name="psD", bufs=4, space="PSUM") as PSD:
                for ki in range(CT):
                    kwp = PSD.tile([128, C], f32, name="dps")
                    for vi in range(CT):
                        mm(kwp[:], lhsT=KVT[vi][:, ts(ki, 128)], rhs=WoT[vi][:],
                           start=(vi == 0), stop=(vi == 3))
                    nc.any.tensor_copy(KVW[ki][:], kwp[:])

                # Mw[m] = (Wgq^T @ KVW) block m; rb = bgq @ KVW + bo
                for m in range(CT):
                    mps2 = PSD.tile([128, C], f32, name="dps")
                    for kk in range(CT):
                        mm(mps2[:], lhsT=wgq_sb[:, ds(kk * 512 + m * 128, 128)],
                           rhs=KVW[kk][:], start=(kk == 0), stop=(kk == 3))
                    nc.any.tensor_copy(Mw[m][:], mps2[:])
                rps = PSD.tile([1, C], f32, name="dps")
                for ki in range(CT):
                    mm(rps[:], lhsT=bgqT[:, ki:ki + 1], rhs=KVW[ki][:],
                       start=(ki == 0), stop=False)
                mm(rps[:], lhsT=ones_bf[:, 0:1], rhs=bo_sb[:],
                   start=False, stop=True)
                nc.any.tensor_copy(rb[:], rps[:])

                # out chunks: z = x @ Mw + rb, stored 2 chunks per DMA
                for g in range(8):
                    zt2 = SBZ.tile([128, 2 * C], f32, name="zt2")
                    for j in range(2):
                        tj = g * 2 + j
                        zps = PSD.tile([128, C], f32, name="dps")
                        for ci in range(CT):
                            mm(zps[:], lhsT=xT[ci][:, ts(tj, 128)], rhs=Mw[ci][:],
                               start=(ci == 0), stop=False)
                        mm(zps[:], lhsT=ones_bf[:], rhs=rb[:],
                           start=False, stop=True)
                        nc.any.tensor_copy(zt2[:, ds(j * 512, 512)], zps[:])
                    nc.sync.dma_start(
                        out=out[g * 256:(g + 1) * 256, :].rearrange(
                            "(j p) c -> p j c", p=128),
                        in_=zt2[:].rearrange("p (j c) -> p j c", j=2),
                    )

    nc.finalize()
    return nc


def _get_nc():
    if "nc" not in _CACHE:
        _CACHE["nc"] = _build()
    return _CACHE["nc"]


def _in_maps(inputs):
    def f(a):
        return np.ascontiguousarray(np.asarray(a, dtype=np.float32))

    x = f(inputs["x"])
    shared = {k: f(inputs[k]) for k in ("Wg", "bg", "Wlr", "blr", "Wo", "bo")}
    return [{"x": x[i], **shared} for i in range(N)]


def _run(in_maps, **kw):
    from concourse.bass_utils import run_bass_kernel_spmd

    nc = _get_nc()
    return run_bass_kernel_spmd(nc, in_maps, list(range(N)), **kw)


def kernel(**inputs) -> np.ndarray:
    res = _run(_in_maps(inputs))
    return np.stack([res.results[i]["out"] for i in range(N)]).astype(np.float32)



# revision 51
# speedup vs baseline: 1.0155x; 1.0155x over previous
# Bass/Trainium2 kernel for nn_Delta (DeltaNet-style recurrence).
#
# Problem (hardcoded): N=8, T=2048, C=512, fp32 I/O.
#   g = x @ Wg.T + bg ; q,k,v = split(g) ; lr = x @ Wlr.T + blr
#   k = k / ||k||
#   delta-rule scan:  u_t = v_t - k_t @ S ; S += outer(k_t, u_t)
#   kv = sum_t k_t (x) (lr_t * u_t) ; y = q @ kv ; out = y @ Wo.T + bo
#
# Sharding: data-parallel over N across the 8 cores (sample i -> core i),
# weights replicated. No collectives.
#
# Per-core algorithm: chunked parallel delta rule with chunk L=128.
#   Within a chunk: (I + A) U = V - K @ S0, A = tril(K K^T, -1).
#   (I+A)^-1 = (I+F1)(I+F2)(I+F3)(I+F4) with commuting factors built from
#   powers of B = -A (base-4 digits):
#     F1 = B+B^2+B^3, F2 = B^4+B^8+B^12, F3 = B^16+B^32+B^48, F4 = B^64.
#   Powers are maintained as (upper, lower) transpose pairs so every matmul
#   has its lhsT available without explicit transposes.
#
# Output path uses associativity to avoid materializing q:
#   y Wo^T = q @ kv @ Wo^T = x @ (Wgq^T kv Wo^T) + bgq @ (kv Wo^T)
# so the T-sized projections are only x->K, x->V, x->out.
#
# Layout strategy: a few large cast-DMA loads (f32->bf16), all transposes
# done on the TensorEngine via identity matmuls (no DMA transposes). The
# power-chain PSUM pools are declared first so they land on banks released
# early, letting each chunk's (state-independent) inverse factors compute
# during the projection phase; the serial solve then streams with the
# factors already in SBUF.

import numpy as np

N, T, C = 8, 2048, 512
L = 128
NCH = T // L  # 16 chunks
CT = C // 128  # 4 c-tiles

_CACHE = {}


def _build():
    import concourse.bacc as bacc
    import concourse.mybir as mybir
    import concourse.tile as tile
    from concourse.bass import ts, ds
    from concourse.masks import (
        make_identity,
        make_lower_triangular,
        make_upper_triangular,
    )

    f32 = mybir.dt.float32
    f32r = mybir.dt.float32r
    bf16 = mybir.dt.bfloat16
    AF = mybir.ActivationFunctionType

    nc = bacc.Bacc("TRN2")
    x = nc.declare_dram_parameter("x", [T, C], f32, isOutput=False)
    Wg = nc.declare_dram_parameter("Wg", [3 * C, C], f32, isOutput=False)
    bg = nc.declare_dram_parameter("bg", [3 * C], f32, isOutput=False)
    Wlr = nc.declare_dram_parameter("Wlr", [1, C], f32, isOutput=False)
    blr = nc.declare_dram_parameter("blr", [1], f32, isOutput=False)
    Wo = nc.declare_dram_parameter("Wo", [C, C], f32, isOutput=False)
    bo = nc.declare_dram_parameter("bo", [C], f32, isOutput=False)
    out = nc.declare_dram_parameter("out", [T, C], f32, isOutput=True)

    mm = nc.tensor.matmul

    with tile.TileContext(nc) as tc:
        with tc.tile_pool(name="persist", bufs=1) as P:
            # ---- constants / small tensors ----
            maskU = P.tile([128, 128], f32, name="maskU")
            maskL = P.tile([128, 128], f32, name="maskL")
            ones_bf = P.tile([1, 128], bf16, name="ones_bf")
            nc.vector.memset(ones_bf[:], 1.0)
            ident_bf = P.tile([128, 128], bf16, name="ident_bf")
            maskU2 = P.tile([128, 256], f32, name="maskU2")
            maskL2 = P.tile([128, 256], f32, name="maskL2")
            ident_f32 = P.tile([128, 128], f32, name="ident_f32")
            ident_fr = P.tile([128, 128], f32r, name="ident_fr")

            # ---- persistent tensors ----
            xT = [P.tile([128, T], bf16, name=f"xT{i}") for i in range(CT)]
            WgTk = [P.tile([128, C], bf16, name=f"WgTk{i}") for i in range(CT)]
            WgTv = [P.tile([128, C], bf16, name=f"WgTv{i}") for i in range(CT)]
            WoT = [P.tile([128, C], bf16, name=f"WoT{i}") for i in range(CT)]
            wgq_sb = P.tile([128, 4 * C], bf16, name="wgq_sb")
            Khat = [P.tile([128, C], f32r, name=f"Khat{i}") for i in range(NCH)]
            # chunk-major K^T: KTc[i][:, ci*128:(ci+1)*128] = Khat_i^T c-block
            KTc = [P.tile([128, C], bf16, name=f"KTc{i}") for i in range(NCH)]
            U = [P.tile([128, C], f32r, name=f"U{i}") for i in range(NCH)]
            # Sn = -S (negated state, bf16 accumulation) so the K@S term
            # accumulates positively in the solve
            S_bf = [P.tile([128, C], bf16, name=f"Sb{i}") for i in range(CT)]
            KVT = [P.tile([128, C], bf16, name=f"KVT{i}") for i in range(CT)]
            KVW = [P.tile([128, C], bf16, name=f"KVW{i}") for i in range(CT)]
            Mw = [P.tile([128, C], bf16, name=f"Mw{i}") for i in range(CT)]
            rb = P.tile([1, C], f32, name="rb")
            rb_bc = P.tile([128, C], f32, name="rb_bc")
            blr_bc = P.tile([128, 1], f32, name="blr_bc")
            lrn = P.tile([128, NCH], f32, name="lrn")
            KVf = [P.tile([128, C], f32, name=f"KVf{i}") for i in range(CT)]

            # ============ phase A: loads + PE transposes + K projections ====
            with tc.tile_pool(name="stage", bufs=1) as STG, \
                 tc.tile_pool(name="sbB", bufs=3) as SBB, \
                 tc.tile_pool(name="small", bufs=6) as SMALL, \
                 tc.tile_pool(name="psTR", bufs=2, space="PSUM") as TRP, \
                 tc.tile_pool(name="psTK", bufs=1, space="PSUM") as TRK, \
                 tc.tile_pool(name="psB", bufs=2, space="PSUM") as PSB, \
                 tc.tile_pool(name="psLr", bufs=1, space="PSUM") as PSLR:
                # large cast loads, rearranged so row-chunk j lands in
                # column block j: sb[p, j*512 + c] = src[j*128 + p, c]
                def big_load(dst, src_rows):
                    nc.gpsimd.dma_start(
                        out=dst[:].rearrange("p (j c) -> p j c", j=4),
                        in_=src_rows.rearrange("(j p) c -> p j c", p=128),
                    )

                x_sb = [STG.tile([128, 4 * C], bf16, name=f"x_sb{g}")
                        for g in range(4)]
                wgk_sb = STG.tile([128, 4 * C], bf16, name="wgk_sb")
                big_load(wgk_sb, Wg[C:2 * C, :])
                make_identity(nc, ident_bf[:])
                big_load(x_sb[0], x[0:512, :])
                wgv_sb = STG.tile([128, 4 * C], bf16, name="wgv_sb")
                big_load(wgv_sb, Wg[2 * C:3 * C, :])
                for g in range(1, 4):
                    big_load(x_sb[g], x[g * 512:(g + 1) * 512, :])
                big_load(wgq_sb, Wg[0:C, :])
                wo_sb = STG.tile([128, 4 * C], bf16, name="wo_sb")
                big_load(wo_sb, Wo[:, :])

                bgk_sb = P.tile([1, C], bf16, name="bgk_sb")
                nc.gpsimd.dma_start(out=bgk_sb[:], in_=bg[C:2 * C])
                bgv_sb = P.tile([1, C], bf16, name="bgv_sb")
                nc.gpsimd.dma_start(out=bgv_sb[:], in_=bg[2 * C:3 * C])
                bo_sb = P.tile([1, C], bf16, name="bo_sb")
                nc.gpsimd.dma_start(out=bo_sb[:], in_=bo[:])
                bgqT = P.tile([128, CT], bf16, name="bgqT")
                nc.gpsimd.dma_start(
                    out=bgqT[:], in_=bg[0:C].rearrange("(i p) -> p i", p=128)
                )
                WlrT = P.tile([128, CT], bf16, name="WlrT")
                nc.gpsimd.dma_start(
                    out=WlrT[:], in_=Wlr[0, :].rearrange("(i p) -> p i", p=128)
                )
                nc.gpsimd.dma_start(
                    out=blr_bc[:],
                    in_=blr[:].rearrange("(o c) -> o c", o=1)
                    .to_broadcast((128, 1)),
                )

                # masks / remaining identities (needed from ~25us on)
                make_upper_triangular(nc, maskU[:], val=-1.0, diag=False)
                make_lower_triangular(nc, maskL[:], val=-1.0, diag=False)
                nc.vector.tensor_copy(maskU2[:, 0:128], maskU[:])
                nc.vector.tensor_copy(maskU2[:, 128:256], maskU[:])
                nc.vector.tensor_copy(maskL2[:, 0:128], maskL[:])
                nc.vector.tensor_copy(maskL2[:, 128:256], maskL[:])
                make_identity(nc, ident_f32[:])
                nc.vector.tensor_copy(ident_fr[:], ident_f32[:])

                # PE transposes: [128,128] tiles via identity matmul.
                def transpose_tiles(src, ci):
                    tp = TRP.tile([128, 4 * 128], bf16, name="tp")
                    for j in range(4):
                        nc.tensor.transpose(
                            tp[:, ts(j, 128)],
                            src[:, ds(j * 512 + ci * 128, 128)],
                            ident_bf[:],
                        )
                    return tp

                for ci in range(CT):
                    tp = transpose_tiles(wgk_sb, ci)
                    nc.any.tensor_copy(WgTk[ci][:], tp[:])
                for ci in range(CT):
                    tp = transpose_tiles(x_sb[0], ci)
                    nc.any.tensor_copy(xT[ci][:, ds(0, 512)], tp[:])
                for ci in range(CT):
                    tp = transpose_tiles(wgv_sb, ci)
                    nc.any.tensor_copy(WgTv[ci][:], tp[:])
                for g in range(1, 4):
                    for ci in range(CT):
                        tp = transpose_tiles(x_sb[g], ci)
                        nc.any.tensor_copy(xT[ci][:, ds(g * 512, 512)], tp[:])
                for ci in range(CT):
                    tp = transpose_tiles(wo_sb, ci)
                    nc.any.tensor_copy(WoT[ci][:], tp[:])

                # K projection + normalization; K^T per chunk (bf16)
                for tj in range(NCH):
                    kps = PSB.tile([128, C], f32, name="kps")
                    for ci in range(CT):
                        mm(kps[:], lhsT=xT[ci][:, ts(tj, 128)], rhs=WgTk[ci][:],
                           start=(ci == 0), stop=False)
                    mm(kps[:], lhsT=ones_bf[:], rhs=bgk_sb[:], start=False,
                       stop=True)
                    sq = SBB.tile([128, C], f32, name="sq")
                    n2 = SMALL.tile([128, 1], f32, name="n2")
                    nc.scalar.activation(sq[:], kps[:], AF.Square, accum_out=n2[:])
                    nrm = SMALL.tile([128, 1], f32, name="nrm")
                    nc.scalar.sqrt(nrm[:], n2[:])
                    rn = SMALL.tile([128, 1], f32, name="rn")
                    nc.vector.reciprocal(rn[:], nrm[:])
                    nc.vector.tensor_scalar_mul(Khat[tj][:], kps[:], rn[:])
                    ktp = TRK.tile([128, C], f32r, name="ktp")
                    for ci in range(CT):
                        nc.tensor.transpose(
                            ktp[:, ts(ci, 128)],
                            Khat[tj][:, ts(ci, 128)],
                            ident_fr[:],
                        )
                    nc.any.tensor_copy(KTc[tj][:], ktp[:])

                # lr column per chunk: lrn[:, i] = x_chunk @ Wlr^T + blr
                lps = PSLR.tile([128, NCH], f32, name="lps")
                for i in range(NCH):
                    for ci in range(CT):
                        mm(lps[:, i:i + 1], lhsT=xT[ci][:, ts(i, 128)],
                           rhs=WlrT[:, ci:ci + 1], start=(ci == 0),
                           stop=(ci == 3))
                nc.any.tensor_scalar_add(lrn[:], lps[:], blr_bc[:])

            # ================= phase C: delta-rule recurrence ===============
            # psPa/psPb are declared first so they are assigned the banks
            # released earliest by phase A, letting the per-chunk inverse
            # factors (independent of the recurrence state) run ahead.
            with tc.tile_pool(name="sbP", bufs=3) as SBP, \
                 tc.tile_pool(name="sbF", bufs=3) as SBF, \
                 tc.tile_pool(name="sbM", bufs=NCH // 2) as SBM, \
                 tc.tile_pool(name="sbG", bufs=18) as SBG, \
                 tc.tile_pool(name="sbU", bufs=2) as SBU, \
                 tc.tile_pool(name="sbLR", bufs=6) as SBLR, \
                 tc.tile_pool(name="psPa", bufs=1, space="PSUM") as PSPA, \
                 tc.tile_pool(name="psPb", bufs=1, space="PSUM") as PSPB, \
                 tc.tile_pool(name="psU", bufs=4, space="PSUM") as PSU, \
                 tc.tile_pool(name="psSD", bufs=2, space="PSUM") as PSSD:
                for ci in range(CT):
                    nc.gpsimd.memset(S_bf[ci][:], 0.0)
                for vi in range(CT):
                    nc.gpsimd.memset(KVf[vi][:], 0.0)

                gxns = {}
                lrus = {}
                Mps = {}
                for i in range(NCH):
                    if i % 2 == 0:
                        # ---- lockstep inverse-factor chain for the chunk
                        # pair (a, b) = (i, i+1). Region layout of every
                        # level tile: {X_a | X_b | Xl_a | Xl_b} so each
                        # level is ONE [128,512] bank with ONE evacuation.
                        a, b = i, i + 1
                        PSP = PSPA if (i // 2) % 2 == 0 else PSPB
                        gps = PSP.tile([128, 256], f32, name="pp")
                        for ci in range(CT):
                            ka = KTc[a][:, ts(ci, 128)]
                            mm(gps[:, 0:128], lhsT=ka, rhs=ka,
                               start=(ci == 0), stop=(ci == 3))
                        for ci in range(CT):
                            kb = KTc[b][:, ts(ci, 128)]
                            mm(gps[:, 128:256], lhsT=kb, rhs=kb,
                               start=(ci == 0), stop=(ci == 3))
                        BuP = SBP.tile([128, 256], bf16, name="BuP")
                        BlP = SBP.tile([128, 256], bf16, name="BlP")
                        nc.any.tensor_mul(BuP[:], gps[:], maskU2[:])
                        nc.any.tensor_mul(BlP[:], gps[:], maskL2[:])

                        def pair2(prev_u, prev_l, name):
                            # squares both chunks: out_u = l^T u, out_l = u^T l
                            ps = PSP.tile([128, 512], f32, name="pp")
                            for s in range(2):
                                pu = prev_u[:, ts(s, 128)]
                                pl = prev_l[:, ts(s, 128)]
                                mm(ps[:, ds(s * 128, 128)], lhsT=pl, rhs=pu,
                                   start=True, stop=True)
                                mm(ps[:, ds(256 + s * 128, 128)], lhsT=pu,
                                   rhs=pl, start=True, stop=True)
                            t = SBP.tile([128, 512], bf16, name=name)
                            if (i // 2) % 2 == 0:
                                nc.vector.tensor_copy(t[:], ps[:])
                            else:
                                nc.scalar.activation(t[:], ps[:], AF.Identity)
                            return t[:, 0:256], t[:, 256:512]

                        B2uP, B2lP = pair2(BuP, BlP, "B22")
                        B4uP, B4lP = pair2(B2uP, B2lP, "B44")
                        B8uP, B8lP = pair2(B4uP, B4lP, "B88")
                        B16uP, B16lP = pair2(B8uP, B8lP, "B1616")
                        B32uP, B32lP = pair2(B16uP, B16lP, "B3232")

                        # fp1X: {B3u_a | B3u_b | B48u_a | B48u_b}
                        fp1 = PSP.tile([128, 512], f32, name="pp")
                        for s in range(2):
                            mm(fp1[:, ds(s * 128, 128)],
                               lhsT=BlP[:, ts(s, 128)], rhs=BuP[:, ts(s, 128)],
                               start=True, stop=False)
                            mm(fp1[:, ds(s * 128, 128)],
                               lhsT=B2lP[:, ts(s, 128)], rhs=BuP[:, ts(s, 128)],
                               start=False, stop=True)
                            mm(fp1[:, ds(256 + s * 128, 128)],
                               lhsT=B8lP[:, ts(s, 128)], rhs=B8uP[:, ts(s, 128)],
                               start=True, stop=False)
                            mm(fp1[:, ds(256 + s * 128, 128)],
                               lhsT=B16lP[:, ts(s, 128)], rhs=B32uP[:, ts(s, 128)],
                               start=False, stop=True)
                        F1P = SBF.tile([128, 256], bf16, name="F1P")
                        nc.any.tensor_add(F1P[:], fp1[:, 0:256], BuP[:])
                        F3P = SBF.tile([128, 256], bf16, name="F3P")
                        nc.any.tensor_add(F3P[:], fp1[:, 256:512], B32uP)

                        # fp2X: {B12u_a | B12u_b | B12l_a | B12l_b}
                        fp2 = PSP.tile([128, 512], f32, name="pp")
                        for s in range(2):
                            mm(fp2[:, ds(s * 128, 128)],
                               lhsT=B2lP[:, ts(s, 128)], rhs=B2uP[:, ts(s, 128)],
                               start=True, stop=False)
                            mm(fp2[:, ds(s * 128, 128)],
                               lhsT=B4lP[:, ts(s, 128)], rhs=B8uP[:, ts(s, 128)],
                               start=False, stop=True)
                            mm(fp2[:, ds(256 + s * 128, 128)],
                               lhsT=B2uP[:, ts(s, 128)], rhs=B2lP[:, ts(s, 128)],
                               start=True, stop=False)
                            mm(fp2[:, ds(256 + s * 128, 128)],
                               lhsT=B8uP[:, ts(s, 128)], rhs=B4lP[:, ts(s, 128)],
                               start=False, stop=True)
                        F2P = SBF.tile([128, 256], bf16, name="F2P")
                        nc.any.tensor_add(F2P[:], fp2[:, 0:256], B8uP)
                        F2lP = SBF.tile([128, 256], bf16, name="F2lP")
                        nc.any.tensor_add(F2lP[:], fp2[:, 256:512], B8lP)

                        # ps64X: {B64u_a | B64u_b | B64l_a | B64l_b}
                        ps64 = PSP.tile([128, 512], f32, name="pp")
                        for s in range(2):
                            mm(ps64[:, ds(s * 128, 128)],
                               lhsT=B32lP[:, ts(s, 128)], rhs=B32uP[:, ts(s, 128)],
                               start=True, stop=True)
                            mm(ps64[:, ds(256 + s * 128, 128)],
                               lhsT=B32uP[:, ts(s, 128)], rhs=B32lP[:, ts(s, 128)],
                               start=True, stop=True)
                        F44P = SBF.tile([128, 512], bf16, name="F44P")
                        if (i // 2) % 2 == 0:
                            nc.scalar.activation(F44P[:], ps64[:], AF.Identity)
                        else:
                            nc.vector.tensor_copy(F44P[:], ps64[:])
                        F4uP, F4lP = F44P[:, 0:256], F44P[:, 256:512]

                        # mpsX: {M12u_a | M12u_b | M34u_a | M34u_b}
                        mps = PSP.tile([128, 512], f32, name="pp")
                        for s in range(2):
                            mm(mps[:, ds(s * 128, 128)],
                               lhsT=F2lP[:, ts(s, 128)], rhs=F1P[:, ts(s, 128)],
                               start=True, stop=False)
                            mm(mps[:, ds(s * 128, 128)],
                               lhsT=ident_bf[:], rhs=F1P[:, ts(s, 128)],
                               start=False, stop=False)
                            mm(mps[:, ds(s * 128, 128)],
                               lhsT=ident_bf[:], rhs=F2P[:, ts(s, 128)],
                               start=False, stop=True)
                            mm(mps[:, ds(256 + s * 128, 128)],
                               lhsT=F4lP[:, ts(s, 128)], rhs=F3P[:, ts(s, 128)],
                               start=True, stop=False)
                            mm(mps[:, ds(256 + s * 128, 128)],
                               lhsT=ident_bf[:], rhs=F3P[:, ts(s, 128)],
                               start=False, stop=False)
                            mm(mps[:, ds(256 + s * 128, 128)],
                               lhsT=ident_bf[:], rhs=F4uP[:, ts(s, 128)],
                               start=False, stop=True)
                        MpP = SBM.tile([128, 512], bf16, name="Mp")
                        nc.any.tensor_copy(MpP[:], mps[:])
                        Mps[i // 2] = MpP

                    # cross-chunk correction matrices within the quad
                    # (independent of U/S; can run ahead):
                    # gxns[src] with gxn^T @ U_src = -K_i K_src^T U_src
                    for src in range(i - i % 2, i):
                        gx = (PSPA if i % 2 == 0 else PSPB).tile(
                            [128, 256], f32, name="pp")
                        for ci in range(CT):
                            mm(gx[:, 0:128], lhsT=KTc[src][:, ts(ci, 128)],
                               rhs=KTc[i][:, ts(ci, 128)],
                               start=(ci == 0), stop=(ci == 3))
                        g_t = SBG.tile([128, 128], f32r, name="gxn")
                        nc.vector.tensor_scalar_mul(g_t[:], gx[:, 0:128], -1.0)
                        gxns[(i, src)] = g_t

                    # --- U_i = (I+F1)(I+F2)(I+F3)(I+F4) (V_i - Khat_i S0) ---
                    # Chunks are paired: odd chunk i reads the pair-start state
                    # S0 plus an explicit cross term -A21 @ U_{i-1}.
                    ups = PSU.tile([128, C], f32, name="ups")
                    for ci in range(CT):
                        mm(ups[:], lhsT=xT[ci][:, ts(i, 128)],
                           rhs=WgTv[ci][:],
                           start=(ci == 0), stop=False)
                    mm(ups[:], lhsT=ones_bf[:], rhs=bgv_sb[:],
                       start=False, stop=(i <= 1))
                    if i >= 2:
                        for ci in range(CT):
                            mm(ups[:], lhsT=KTc[i][:, ts(ci, 128)],
                               rhs=S_bf[ci][:],
                               start=False, stop=(ci == 3))
                    for src in range(i - i % 2, i):
                        mm(ups[:], lhsT=gxns[(i, src)][:], rhs=U[src][:],
                           start=False, stop=True,
                           skip_group_check=True)
                    MpP = Mps[i // 2]
                    M12u = MpP[:, ds((i % 2) * 128, 128)]
                    M34u = MpP[:, ds(256 + (i % 2) * 128, 128)]
                    ucur = SBU.tile([128, C], bf16, name="ua")
                    nc.any.tensor_copy(ucur[:], ups[:])  # U_0 = V - Khat S0
                    mm(ups[:], lhsT=M34u, rhs=ucur[:], start=False, stop=True,
                       skip_group_check=True)
                    umid = SBU.tile([128, C], bf16, name="ub")
                    nc.any.tensor_copy(umid[:], ups[:])
                    mm(ups[:], lhsT=M12u, rhs=umid[:], start=False, stop=True,
                       skip_group_check=True)
                    nc.any.tensor_copy(U[i][:], ups[:])
                    lru = SBLR.tile([128, C], f32r, name="lru")
                    nc.vector.tensor_scalar_mul(lru[:], U[i][:], lrn[:, i:i + 1])
                    lrus[i] = lru

                    # --- Sn -= Khat^T U for the quad, at quad end (bf16) ---
                    if i % 2 == 1 and i < NCH - 1:
                        for ci in range(CT):
                            sd = PSSD.tile([128, C], f32, name="sd")
                            for j in range(2):
                                mm(sd[:], lhsT=Khat[i - 1 + j][:, ts(ci, 128)],
                                   rhs=U[i - 1 + j][:], start=(j == 0),
                                   stop=(j == 1))
                            nc.vector.tensor_sub(S_bf[ci][:], S_bf[ci][:], sd[:])
                    # --- kv partial: KVf[vi] += sum_j lru_j^T(vi) Khat_j ---
                    if i % 4 == 3:
                        last = (i == NCH - 1)
                        for vi in range(CT):
                            kvp = PSSD.tile([128, C], f32, name="sd")
                            for j in range(4):
                                cj = i - 3 + j
                                mm(kvp[:], lhsT=lrus[cj][:, ts(vi, 128)],
                                   rhs=Khat[cj][:], start=(j == 0),
                                   stop=(j == 3))
                            if last:
                                # final partial: write the bf16 KVT directly
                                nc.vector.tensor_add(KVT[vi][:], kvp[:],
                                                     KVf[vi][:])
                            else:
                                nc.vector.tensor_add(KVf[vi][:], kvp[:],
                                                     KVf[vi][:])
                        lrus.clear()

            # ================= phase D: outputs =============================
            with tc.tile_pool(name="sbD", bufs=4) as SBD, \
                 tc.tile_pool(name="sbZ", bufs=3) as SBZ, \
                 tc.tile_pool(name="psD", bufs=4, space="PSUM") as PSD:
                for ki in range(CT):
                    kwp = PSD.tile([128, C], f32, name="dps")
                    for vi in range(CT):
                        mm(kwp[:], lhsT=KVT[vi][:, ts(ki, 128)], rhs=WoT[vi][:],
                           start=(vi == 0), stop=(vi == 3))
                    nc.any.tensor_copy(KVW[ki][:], kwp[:])

                # Mw[m] = (Wgq^T @ KVW) block m; rb = bgq @ KVW + bo
                for m in range(CT):
                    mps2 = PSD.tile([128, C], f32, name="dps")
                    for kk in range(CT):
                        mm(mps2[:], lhsT=wgq_sb[:, ds(kk * 512 + m * 128, 128)],
                           rhs=KVW[kk][:], start=(kk == 0), stop=(kk == 3))
                    nc.any.tensor_copy(Mw[m][:], mps2[:])
                rps = PSD.tile([1, C], f32, name="dps")
                for ki in range(CT):
                    mm(rps[:], lhsT=bgqT[:, ki:ki + 1], rhs=KVW[ki][:],
                       start=(ki == 0), stop=False)
                mm(rps[:], lhsT=ones_bf[:, 0:1], rhs=bo_sb[:],
                   start=False, stop=True)
                nc.any.tensor_copy(rb[:], rps[:])
                nc.gpsimd.partition_broadcast(rb_bc[:], rb[:])

                # out chunks: z = x @ Mw + rb, stored 2 chunks per DMA
                for g in range(8):
                    zt2 = SBZ.tile([128, 2 * C], f32, name="zt2")
                    for j in range(2):
                        tj = g * 2 + j
                        zps = PSD.tile([128, C], f32, name="dps")
                        for ci in range(CT):
                            mm(zps[:], lhsT=xT[ci][:, ts(tj, 128)], rhs=Mw[ci][:],
                               start=(ci == 0), stop=(ci == 3))
                        nc.any.tensor_add(zt2[:, ds(j * 512, 512)], zps[:],
                                          rb_bc[:])
                    nc.sync.dma_start(
                        out=out[g * 256:(g + 1) * 256, :].rearrange(
                            "(j p) c -> p j c", p=128),
                        in_=zt2[:].rearrange("p (j c) -> p j c", j=2),
                    )

    nc.finalize()
    return nc


def _get_nc():
    if "nc" not in _CACHE:
        _CACHE["nc"] = _build()
    return _CACHE["nc"]


def _in_maps(inputs):
    def f(a):
        return np.ascontiguousarray(np.asarray(a, dtype=np.float32))

    x = f(inputs["x"])
    shared = {k: f(inputs[k]) for k in ("Wg", "bg", "Wlr", "blr", "Wo", "bo")}
    return [{"x": x[i], **shared} for i in range(N)]


def _run(in_maps, **kw):
    from concourse.bass_utils import run_bass_kernel_spmd

    nc = _get_nc()
    return run_bass_kernel_spmd(nc, in_maps, list(range(N)), **kw)


def kernel(**inputs) -> np.ndarray:
    res = _run(_in_maps(inputs))
    return np.stack([res.results[i]["out"] for i in range(N)]).astype(np.float32)



# revision 60
# speedup vs baseline: 1.0310x; 1.0153x over previous
# Bass/Trainium2 kernel for nn_Delta (DeltaNet-style recurrence).
#
# Problem (hardcoded): N=8, T=2048, C=512, fp32 I/O.
#   g = x @ Wg.T + bg ; q,k,v = split(g) ; lr = x @ Wlr.T + blr
#   k = k / ||k||
#   delta-rule scan:  u_t = v_t - k_t @ S ; S += outer(k_t, u_t)
#   kv = sum_t k_t (x) (lr_t * u_t) ; y = q @ kv ; out = y @ Wo.T + bo
#
# Sharding: data-parallel over N across the 8 cores (sample i -> core i),
# weights replicated. No collectives.
#
# Per-core algorithm: chunked parallel delta rule with chunk L=128.
#   Within a chunk: (I + A) U = V - K @ S0, A = tril(K K^T, -1).
#   (I+A)^-1 = (I+F1)(I+F2)(I+F3)(I+F4) with commuting factors built from
#   powers of B = -A (base-4 digits):
#     F1 = B+B^2+B^3, F2 = B^4+B^8+B^12, F3 = B^16+B^32+B^48, F4 = B^64.
#   Powers are maintained as (upper, lower) transpose pairs so every matmul
#   has its lhsT available without explicit transposes.
#
# Output path uses associativity to avoid materializing q:
#   y Wo^T = q @ kv @ Wo^T = x @ (Wgq^T kv Wo^T) + bgq @ (kv Wo^T)
# so the T-sized projections are only x->K, x->V, x->out.
#
# Layout strategy: a few large cast-DMA loads (f32->bf16), all transposes
# done on the TensorEngine via identity matmuls (no DMA transposes). The
# power-chain PSUM pools are declared first so they land on banks released
# early, letting each chunk's (state-independent) inverse factors compute
# during the projection phase; the serial solve then streams with the
# factors already in SBUF.

import numpy as np

N, T, C = 8, 2048, 512
L = 128
NCH = T // L  # 16 chunks
CT = C // 128  # 4 c-tiles

_CACHE = {}


def _build():
    import concourse.bacc as bacc
    import concourse.mybir as mybir
    import concourse.tile as tile
    from concourse.bass import ts, ds
    from concourse.masks import (
        make_identity,
        make_lower_triangular,
        make_upper_triangular,
    )

    f32 = mybir.dt.float32
    f32r = mybir.dt.float32r
    bf16 = mybir.dt.bfloat16
    AF = mybir.ActivationFunctionType

    nc = bacc.Bacc("TRN2")
    x = nc.declare_dram_parameter("x", [T, C], f32, isOutput=False)
    Wg = nc.declare_dram_parameter("Wg", [3 * C, C], f32, isOutput=False)
    bg = nc.declare_dram_parameter("bg", [3 * C], f32, isOutput=False)
    Wlr = nc.declare_dram_parameter("Wlr", [1, C], f32, isOutput=False)
    blr = nc.declare_dram_parameter("blr", [1], f32, isOutput=False)
    Wo = nc.declare_dram_parameter("Wo", [C, C], f32, isOutput=False)
    bo = nc.declare_dram_parameter("bo", [C], f32, isOutput=False)
    out = nc.declare_dram_parameter("out", [T, C], f32, isOutput=True)

    mm = nc.tensor.matmul

    with tile.TileContext(nc) as tc:
        with tc.tile_pool(name="persist", bufs=1) as P:
            # ---- constants / small tensors ----
            maskU = P.tile([128, 128], f32, name="maskU")
            maskL = P.tile([128, 128], f32, name="maskL")
            ones_bf = P.tile([1, 128], bf16, name="ones_bf")
            nc.vector.memset(ones_bf[:], 1.0)
            ident_bf = P.tile([128, 128], bf16, name="ident_bf")
            maskU2 = P.tile([128, 256], f32, name="maskU2")
            maskL2 = P.tile([128, 256], f32, name="maskL2")
            ident_f32 = P.tile([128, 128], f32, name="ident_f32")
            ident_fr = P.tile([128, 128], f32r, name="ident_fr")

            # ---- persistent tensors ----
            xT = [P.tile([128, T], bf16, name=f"xT{i}") for i in range(CT)]
            WgTk = [P.tile([128, C], bf16, name=f"WgTk{i}") for i in range(CT)]
            WgTv = [P.tile([128, C], bf16, name=f"WgTv{i}") for i in range(CT)]
            WoT = [P.tile([128, C], bf16, name=f"WoT{i}") for i in range(CT)]
            wgq_sb = P.tile([128, 4 * C], bf16, name="wgq_sb")
            Khat = [P.tile([128, C], f32r, name=f"Khat{i}") for i in range(NCH)]
            # chunk-major K^T: KTc[i][:, ci*128:(ci+1)*128] = Khat_i^T c-block
            KTc = [P.tile([128, C], bf16, name=f"KTc{i}") for i in range(NCH)]
            U = [P.tile([128, C], f32r, name=f"U{i}") for i in range(NCH)]
            # Sn = -S (negated state, bf16 accumulation) so the K@S term
            # accumulates positively in the solve
            S_bf = [P.tile([128, C], bf16, name=f"Sb{i}") for i in range(CT)]
            KVT = [P.tile([128, C], bf16, name=f"KVT{i}") for i in range(CT)]
            KVW = [P.tile([128, C], bf16, name=f"KVW{i}") for i in range(CT)]
            Mw = [P.tile([128, C], bf16, name=f"Mw{i}") for i in range(CT)]
            rb = P.tile([1, C], f32, name="rb")
            rb_bc = P.tile([128, C], f32, name="rb_bc")
            blr_bc = P.tile([128, 1], f32, name="blr_bc")
            lrn = P.tile([128, NCH], f32, name="lrn")
            KVf = [P.tile([128, C], f32, name=f"KVf{i}") for i in range(CT)]

            # ============ phase A: loads + PE transposes + K projections ====
            with tc.tile_pool(name="stage", bufs=1) as STG, \
                 tc.tile_pool(name="sbB", bufs=3) as SBB, \
                 tc.tile_pool(name="small", bufs=6) as SMALL, \
                 tc.tile_pool(name="psTR", bufs=3, space="PSUM") as TRP, \
                 tc.tile_pool(name="psTK", bufs=1, space="PSUM") as TRK, \
                 tc.tile_pool(name="psB", bufs=3, space="PSUM") as PSB, \
                 tc.tile_pool(name="psLr", bufs=1, space="PSUM") as PSLR:
                # large cast loads, rearranged so row-chunk j lands in
                # column block j: sb[p, j*512 + c] = src[j*128 + p, c]
                def big_load(dst, src_rows):
                    nc.gpsimd.dma_start(
                        out=dst[:].rearrange("p (j c) -> p j c", j=4),
                        in_=src_rows.rearrange("(j p) c -> p j c", p=128),
                    )

                x_sb = [STG.tile([128, 4 * C], bf16, name=f"x_sb{g}")
                        for g in range(4)]
                wgk_sb = STG.tile([128, 4 * C], bf16, name="wgk_sb")
                big_load(wgk_sb, Wg[C:2 * C, :])
                make_identity(nc, ident_bf[:])
                big_load(x_sb[0], x[0:512, :])
                # small bias loads early: bgk gates the first kproj's stop,
                # bgv the first V-proj; behind all big loads they'd land ~16us
                bgk_sb = P.tile([1, C], bf16, name="bgk_sb")
                nc.gpsimd.dma_start(out=bgk_sb[:], in_=bg[C:2 * C])
                bgv_sb = P.tile([1, C], bf16, name="bgv_sb")
                nc.gpsimd.dma_start(out=bgv_sb[:], in_=bg[2 * C:3 * C])
                wgv_sb = STG.tile([128, 4 * C], bf16, name="wgv_sb")
                big_load(wgv_sb, Wg[2 * C:3 * C, :])
                for g in range(1, 4):
                    big_load(x_sb[g], x[g * 512:(g + 1) * 512, :])
                WlrT = P.tile([128, CT], bf16, name="WlrT")
                nc.gpsimd.dma_start(
                    out=WlrT[:], in_=Wlr[0, :].rearrange("(i p) -> p i", p=128)
                )
                nc.gpsimd.dma_start(
                    out=blr_bc[:],
                    in_=blr[:].rearrange("(o c) -> o c", o=1)
                    .to_broadcast((128, 1)),
                )
                big_load(wgq_sb, Wg[0:C, :])
                wo_sb = STG.tile([128, 4 * C], bf16, name="wo_sb")
                big_load(wo_sb, Wo[:, :])
                bo_sb = P.tile([1, C], bf16, name="bo_sb")
                nc.gpsimd.dma_start(out=bo_sb[:], in_=bo[:])
                bgqT = P.tile([128, CT], bf16, name="bgqT")
                nc.gpsimd.dma_start(
                    out=bgqT[:], in_=bg[0:C].rearrange("(i p) -> p i", p=128)
                )

                # masks / remaining identities (needed from ~25us on)
                make_upper_triangular(nc, maskU[:], val=-1.0, diag=False)
                make_lower_triangular(nc, maskL[:], val=-1.0, diag=False)
                nc.vector.tensor_copy(maskU2[:, 0:128], maskU[:])
                nc.vector.tensor_copy(maskU2[:, 128:256], maskU[:])
                nc.vector.tensor_copy(maskL2[:, 0:128], maskL[:])
                nc.vector.tensor_copy(maskL2[:, 128:256], maskL[:])
                make_identity(nc, ident_f32[:])
                nc.vector.tensor_copy(ident_fr[:], ident_f32[:])

                # PE transposes: [128,128] tiles via identity matmul.
                def transpose_tiles(src, ci):
                    tp = TRP.tile([128, 4 * 128], bf16, name="tp")
                    for j in range(4):
                        nc.tensor.transpose(
                            tp[:, ts(j, 128)],
                            src[:, ds(j * 512 + ci * 128, 128)],
                            ident_bf[:],
                        )
                    return tp

                for ci in range(CT):
                    tp = transpose_tiles(wgk_sb, ci)
                    nc.any.tensor_copy(WgTk[ci][:], tp[:])
                for ci in range(CT):
                    tp = transpose_tiles(x_sb[0], ci)
                    nc.any.tensor_copy(xT[ci][:, ds(0, 512)], tp[:])
                for ci in range(CT):
                    tp = transpose_tiles(wgv_sb, ci)
                    nc.any.tensor_copy(WgTv[ci][:], tp[:])
                for g in range(1, 4):
                    for ci in range(CT):
                        tp = transpose_tiles(x_sb[g], ci)
                        nc.any.tensor_copy(xT[ci][:, ds(g * 512, 512)], tp[:])
                for ci in range(CT):
                    tp = transpose_tiles(wo_sb, ci)
                    nc.any.tensor_copy(WoT[ci][:], tp[:])

                # K projection + normalization; K^T per chunk (bf16)
                for tj in range(NCH):
                    kps = PSB.tile([128, C], f32, name="kps")
                    for ci in range(CT):
                        mm(kps[:], lhsT=xT[ci][:, ts(tj, 128)], rhs=WgTk[ci][:],
                           start=(ci == 0), stop=False)
                    mm(kps[:], lhsT=ones_bf[:], rhs=bgk_sb[:], start=False,
                       stop=True)
                    sq = SBB.tile([128, C], f32, name="sq")
                    n2 = SMALL.tile([128, 1], f32, name="n2")
                    nc.scalar.activation(sq[:], kps[:], AF.Square, accum_out=n2[:])
                    nrm = SMALL.tile([128, 1], f32, name="nrm")
                    nc.scalar.sqrt(nrm[:], n2[:])
                    rn = SMALL.tile([128, 1], f32, name="rn")
                    nc.vector.reciprocal(rn[:], nrm[:])
                    nc.vector.tensor_scalar_mul(Khat[tj][:], kps[:], rn[:])
                    ktp = TRK.tile([128, C], f32r, name="ktp")
                    for ci in range(CT):
                        nc.tensor.transpose(
                            ktp[:, ts(ci, 128)],
                            Khat[tj][:, ts(ci, 128)],
                            ident_fr[:],
                        )
                    nc.any.tensor_copy(KTc[tj][:], ktp[:])

                # lr column per chunk: lrn[:, i] = x_chunk @ Wlr^T + blr
                lps = PSLR.tile([128, NCH], f32, name="lps")
                for i in range(NCH):
                    for ci in range(CT):
                        mm(lps[:, i:i + 1], lhsT=xT[ci][:, ts(i, 128)],
                           rhs=WlrT[:, ci:ci + 1], start=(ci == 0),
                           stop=(ci == 3))
                nc.any.tensor_scalar_add(lrn[:], lps[:], blr_bc[:])

            # ================= phase C: delta-rule recurrence ===============
            # psPa/psPb are declared first so they are assigned the banks
            # released earliest by phase A, letting the per-chunk inverse
            # factors (independent of the recurrence state) run ahead.
            with tc.tile_pool(name="sbP", bufs=3) as SBP, \
                 tc.tile_pool(name="sbF", bufs=3) as SBF, \
                 tc.tile_pool(name="sbM", bufs=NCH // 2) as SBM, \
                 tc.tile_pool(name="sbG", bufs=18) as SBG, \
                 tc.tile_pool(name="sbU", bufs=2) as SBU, \
                 tc.tile_pool(name="sbLR", bufs=6) as SBLR, \
                 tc.tile_pool(name="psPa", bufs=1, space="PSUM") as PSPA, \
                 tc.tile_pool(name="psPb", bufs=1, space="PSUM") as PSPB, \
                 tc.tile_pool(name="psU", bufs=4, space="PSUM") as PSU, \
                 tc.tile_pool(name="psSD", bufs=2, space="PSUM") as PSSD:
                for ci in range(CT):
                    nc.gpsimd.memset(S_bf[ci][:], 0.0)
                for vi in range(CT):
                    nc.gpsimd.memset(KVf[vi][:], 0.0)

                gxns = {}
                lrus = {}
                Mps = {}
                for i in range(NCH):
                    if i % 2 == 0:
                        # ---- lockstep inverse-factor chain for the chunk
                        # pair (a, b) = (i, i+1). Region layout of every
                        # level tile: {X_a | X_b | Xl_a | Xl_b} so each
                        # level is ONE [128,512] bank with ONE evacuation.
                        a, b = i, i + 1
                        PSP = PSPA if (i // 2) % 2 == 0 else PSPB
                        gps = PSP.tile([128, 256], f32, name="pp")
                        for ci in range(CT):
                            ka = KTc[a][:, ts(ci, 128)]
                            mm(gps[:, 0:128], lhsT=ka, rhs=ka,
                               start=(ci == 0), stop=(ci == 3))
                        for ci in range(CT):
                            kb = KTc[b][:, ts(ci, 128)]
                            mm(gps[:, 128:256], lhsT=kb, rhs=kb,
                               start=(ci == 0), stop=(ci == 3))
                        BuP = SBP.tile([128, 256], bf16, name="BuP")
                        BlP = SBP.tile([128, 256], bf16, name="BlP")
                        nc.any.tensor_mul(BuP[:], gps[:], maskU2[:])
                        nc.any.tensor_mul(BlP[:], gps[:], maskL2[:])

                        def pair2(prev_u, prev_l, name):
                            # squares both chunks: out_u = l^T u, out_l = u^T l
                            ps = PSP.tile([128, 512], f32, name="pp")
                            for s in range(2):
                                pu = prev_u[:, ts(s, 128)]
                                pl = prev_l[:, ts(s, 128)]
                                mm(ps[:, ds(s * 128, 128)], lhsT=pl, rhs=pu,
                                   start=True, stop=True)
                                mm(ps[:, ds(256 + s * 128, 128)], lhsT=pu,
                                   rhs=pl, start=True, stop=True)
                            t = SBP.tile([128, 512], bf16, name=name)
                            if (i // 2) % 2 == 0:
                                nc.vector.tensor_copy(t[:], ps[:])
                            else:
                                nc.scalar.activation(t[:], ps[:], AF.Identity)
                            return t[:, 0:256], t[:, 256:512]

                        B2uP, B2lP = pair2(BuP, BlP, "B22")
                        B4uP, B4lP = pair2(B2uP, B2lP, "B44")
                        B8uP, B8lP = pair2(B4uP, B4lP, "B88")
                        B16uP, B16lP = pair2(B8uP, B8lP, "B1616")
                        B32uP, B32lP = pair2(B16uP, B16lP, "B3232")

                        # fp1X: {B3u_a | B3u_b | B48u_a | B48u_b}
                        fp1 = PSP.tile([128, 512], f32, name="pp")
                        for s in range(2):
                            mm(fp1[:, ds(s * 128, 128)],
                               lhsT=BlP[:, ts(s, 128)], rhs=BuP[:, ts(s, 128)],
                               start=True, stop=False)
                            mm(fp1[:, ds(s * 128, 128)],
                               lhsT=B2lP[:, ts(s, 128)], rhs=BuP[:, ts(s, 128)],
                               start=False, stop=True)
                            mm(fp1[:, ds(256 + s * 128, 128)],
                               lhsT=B8lP[:, ts(s, 128)], rhs=B8uP[:, ts(s, 128)],
                               start=True, stop=False)
                            mm(fp1[:, ds(256 + s * 128, 128)],
                               lhsT=B16lP[:, ts(s, 128)], rhs=B32uP[:, ts(s, 128)],
                               start=False, stop=True)
                        F1P = SBF.tile([128, 256], bf16, name="F1P")
                        nc.any.tensor_add(F1P[:], fp1[:, 0:256], BuP[:])
                        F3P = SBF.tile([128, 256], bf16, name="F3P")
                        nc.any.tensor_add(F3P[:], fp1[:, 256:512], B32uP)

                        # fp2X: {B12u_a | B12u_b | B12l_a | B12l_b}
                        fp2 = PSP.tile([128, 512], f32, name="pp")
                        for s in range(2):
                            mm(fp2[:, ds(s * 128, 128)],
                               lhsT=B2lP[:, ts(s, 128)], rhs=B2uP[:, ts(s, 128)],
                               start=True, stop=False)
                            mm(fp2[:, ds(s * 128, 128)],
                               lhsT=B4lP[:, ts(s, 128)], rhs=B8uP[:, ts(s, 128)],
                               start=False, stop=True)
                            mm(fp2[:, ds(256 + s * 128, 128)],
                               lhsT=B2uP[:, ts(s, 128)], rhs=B2lP[:, ts(s, 128)],
                               start=True, stop=False)
                            mm(fp2[:, ds(256 + s * 128, 128)],
                               lhsT=B8uP[:, ts(s, 128)], rhs=B4lP[:, ts(s, 128)],
                               start=False, stop=True)
                        F2P = SBF.tile([128, 256], bf16, name="F2P")
                        nc.any.tensor_add(F2P[:], fp2[:, 0:256], B8uP)
                        F2lP = SBF.tile([128, 256], bf16, name="F2lP")
                        nc.any.tensor_add(F2lP[:], fp2[:, 256:512], B8lP)

                        # ps64X: {B64u_a | B64u_b | B64l_a | B64l_b}
                        ps64 = PSP.tile([128, 512], f32, name="pp")
                        for s in range(2):
                            mm(ps64[:, ds(s * 128, 128)],
                               lhsT=B32lP[:, ts(s, 128)], rhs=B32uP[:, ts(s, 128)],
                               start=True, stop=True)
                            mm(ps64[:, ds(256 + s * 128, 128)],
                               lhsT=B32uP[:, ts(s, 128)], rhs=B32lP[:, ts(s, 128)],
                               start=True, stop=True)
                        F44P = SBF.tile([128, 512], bf16, name="F44P")
                        if (i // 2) % 2 == 0:
                            nc.scalar.activation(F44P[:], ps64[:], AF.Identity)
                        else:
                            nc.vector.tensor_copy(F44P[:], ps64[:])
                        F4uP, F4lP = F44P[:, 0:256], F44P[:, 256:512]

                        # mpsX: {M12u_a | M12u_b | M34u_a | M34u_b}
                        mps = PSP.tile([128, 512], f32, name="pp")
                        for s in range(2):
                            mm(mps[:, ds(s * 128, 128)],
                               lhsT=F2lP[:, ts(s, 128)], rhs=F1P[:, ts(s, 128)],
                               start=True, stop=False)
                            mm(mps[:, ds(s * 128, 128)],
                               lhsT=ident_bf[:], rhs=F1P[:, ts(s, 128)],
                               start=False, stop=False)
                            mm(mps[:, ds(s * 128, 128)],
                               lhsT=ident_bf[:], rhs=F2P[:, ts(s, 128)],
                               start=False, stop=True)
                            mm(mps[:, ds(256 + s * 128, 128)],
                               lhsT=F4lP[:, ts(s, 128)], rhs=F3P[:, ts(s, 128)],
                               start=True, stop=False)
                            mm(mps[:, ds(256 + s * 128, 128)],
                               lhsT=ident_bf[:], rhs=F3P[:, ts(s, 128)],
                               start=False, stop=False)
                            mm(mps[:, ds(256 + s * 128, 128)],
                               lhsT=ident_bf[:], rhs=F4uP[:, ts(s, 128)],
                               start=False, stop=True)
                        MpP = SBM.tile([128, 512], bf16, name="Mp")
                        nc.any.tensor_copy(MpP[:], mps[:])
                        Mps[i // 2] = MpP

                    # cross-chunk correction matrices within the quad
                    # (independent of U/S; can run ahead):
                    # gxns[src] with gxn^T @ U_src = -K_i K_src^T U_src
                    for src in range(i - i % 2, i):
                        gx = (PSPA if i % 2 == 0 else PSPB).tile(
                            [128, 256], f32, name="pp")
                        for ci in range(CT):
                            mm(gx[:, 0:128], lhsT=KTc[src][:, ts(ci, 128)],
                               rhs=KTc[i][:, ts(ci, 128)],
                               start=(ci == 0), stop=(ci == 3))
                        g_t = SBG.tile([128, 128], f32r, name="gxn")
                        nc.vector.tensor_scalar_mul(g_t[:], gx[:, 0:128], -1.0)
                        gxns[(i, src)] = g_t

                    # --- U_i = (I+F1)(I+F2)(I+F3)(I+F4) (V_i - Khat_i S0) ---
                    # Chunks are paired: odd chunk i reads the pair-start state
                    # S0 plus an explicit cross term -A21 @ U_{i-1}.
                    ups = PSU.tile([128, C], f32, name="ups")
                    for ci in range(CT):
                        mm(ups[:], lhsT=xT[ci][:, ts(i, 128)],
                           rhs=WgTv[ci][:],
                           start=(ci == 0), stop=False)
                    mm(ups[:], lhsT=ones_bf[:], rhs=bgv_sb[:],
                       start=False, stop=(i <= 1))
                    if i >= 2:
                        for ci in range(CT):
                            mm(ups[:], lhsT=KTc[i][:, ts(ci, 128)],
                               rhs=S_bf[ci][:],
                               start=False, stop=(ci == 3))
                    for src in range(i - i % 2, i):
                        mm(ups[:], lhsT=gxns[(i, src)][:], rhs=U[src][:],
                           start=False, stop=True,
                           skip_group_check=True)
                    MpP = Mps[i // 2]
                    M12u = MpP[:, ds((i % 2) * 128, 128)]
                    M34u = MpP[:, ds(256 + (i % 2) * 128, 128)]
                    ucur = SBU.tile([128, C], bf16, name="ua")
                    nc.any.tensor_copy(ucur[:], ups[:])  # U_0 = V - Khat S0
                    mm(ups[:], lhsT=M34u, rhs=ucur[:], start=False, stop=True,
                       skip_group_check=True)
                    umid = SBU.tile([128, C], bf16, name="ub")
                    nc.any.tensor_copy(umid[:], ups[:])
                    mm(ups[:], lhsT=M12u, rhs=umid[:], start=False, stop=True,
                       skip_group_check=True)
                    nc.any.tensor_copy(U[i][:], ups[:])
                    lru = SBLR.tile([128, C], f32r, name="lru")
                    nc.vector.tensor_scalar_mul(lru[:], U[i][:], lrn[:, i:i + 1])
                    lrus[i] = lru

                    # --- Sn -= Khat^T U for the quad, at quad end (bf16) ---
                    if i % 2 == 1 and i < NCH - 1:
                        for ci in range(CT):
                            sd = PSSD.tile([128, C], f32, name="sd")
                            for j in range(2):
                                mm(sd[:], lhsT=Khat[i - 1 + j][:, ts(ci, 128)],
                                   rhs=U[i - 1 + j][:], start=(j == 0),
                                   stop=(j == 1))
                            nc.vector.tensor_sub(S_bf[ci][:], S_bf[ci][:], sd[:])
                    # --- kv partial: KVf[vi] += sum_j lru_j^T(vi) Khat_j ---
                    if i % 4 == 3:
                        last = (i == NCH - 1)
                        for vi in range(CT):
                            kvp = PSSD.tile([128, C], f32, name="sd")
                            for j in range(4):
                                cj = i - 3 + j
                                mm(kvp[:], lhsT=lrus[cj][:, ts(vi, 128)],
                                   rhs=Khat[cj][:], start=(j == 0),
                                   stop=(j == 3))
                            if last:
                                # final partial: write the bf16 KVT directly
                                nc.vector.tensor_add(KVT[vi][:], kvp[:],
                                                     KVf[vi][:])
                            else:
                                nc.vector.tensor_add(KVf[vi][:], kvp[:],
                                                     KVf[vi][:])
                        lrus.clear()

            # ================= phase D: outputs =============================
            with tc.tile_pool(name="sbD", bufs=4) as SBD, \
                 tc.tile_pool(name="sbZ", bufs=3) as SBZ, \
                 tc.tile_pool(name="psD", bufs=4, space="PSUM") as PSD:
                for ki in range(CT):
                    kwp = PSD.tile([128, C], f32, name="dps")
                    for vi in range(CT):
                        mm(kwp[:], lhsT=KVT[vi][:, ts(ki, 128)], rhs=WoT[vi][:],
                           start=(vi == 0), stop=(vi == 3))
                    nc.any.tensor_copy(KVW[ki][:], kwp[:])

                # Mw[m] = (Wgq^T @ KVW) block m; rb = bgq @ KVW + bo
                for m in range(CT):
                    mps2 = PSD.tile([128, C], f32, name="dps")
                    for kk in range(CT):
                        mm(mps2[:], lhsT=wgq_sb[:, ds(kk * 512 + m * 128, 128)],
                           rhs=KVW[kk][:], start=(kk == 0), stop=(kk == 3))
                    nc.any.tensor_copy(Mw[m][:], mps2[:])
                rps = PSD.tile([1, C], f32, name="dps")
                for ki in range(CT):
                    mm(rps[:], lhsT=bgqT[:, ki:ki + 1], rhs=KVW[ki][:],
                       start=(ki == 0), stop=False)
                mm(rps[:], lhsT=ones_bf[:, 0:1], rhs=bo_sb[:],
                   start=False, stop=True)
                nc.any.tensor_copy(rb[:], rps[:])
                nc.gpsimd.partition_broadcast(rb_bc[:], rb[:])

                # out chunks: z = x @ Mw + rb, stored 2 chunks per DMA
                for g in range(8):
                    zt2 = SBZ.tile([128, 2 * C], f32, name="zt2")
                    for j in range(2):
                        tj = g * 2 + j
                        zps = PSD.tile([128, C], f32, name="dps")
                        for ci in range(CT):
                            mm(zps[:], lhsT=xT[ci][:, ts(tj, 128)], rhs=Mw[ci][:],
                               start=(ci == 0), stop=(ci == 3))
                        nc.any.tensor_add(zt2[:, ds(j * 512, 512)], zps[:],
                                          rb_bc[:])
                    nc.sync.dma_start(
                        out=out[g * 256:(g + 1) * 256, :].rearrange(
                            "(j p) c -> p j c", p=128),
                        in_=zt2[:].rearrange("p (j c) -> p j c", j=2),
                    )

    nc.finalize()
    return nc


def _get_nc():
    if "nc" not in _CACHE:
        _CACHE["nc"] = _build()
    return _CACHE["nc"]


def _in_maps(inputs):
    def f(a):
        return np.ascontiguousarray(np.asarray(a, dtype=np.float32))

    x = f(inputs["x"])
    shared = {k: f(inputs[k]) for k in ("Wg", "bg", "Wlr", "blr", "Wo", "bo")}
    return [{"x": x[i], **shared} for i in range(N)]


def _run(in_maps, **kw):
    from concourse.bass_utils import run_bass_kernel_spmd

    nc = _get_nc()
    return run_bass_kernel_spmd(nc, in_maps, list(range(N)), **kw)


def kernel(**inputs) -> np.ndarray:
    res = _run(_in_maps(inputs))
    return np.stack([res.results[i]["out"] for i in range(N)]).astype(np.float32)



# revision 68
# speedup vs baseline: 1.0429x; 1.0116x over previous
# Bass/Trainium2 kernel for nn_Delta (DeltaNet-style recurrence).
#
# Problem (hardcoded): N=8, T=2048, C=512, fp32 I/O.
#   g = x @ Wg.T + bg ; q,k,v = split(g) ; lr = x @ Wlr.T + blr
#   k = k / ||k||
#   delta-rule scan:  u_t = v_t - k_t @ S ; S += outer(k_t, u_t)
#   kv = sum_t k_t (x) (lr_t * u_t) ; y = q @ kv ; out = y @ Wo.T + bo
#
# Sharding: data-parallel over N across the 8 cores (sample i -> core i),
# weights replicated. No collectives.
#
# Per-core algorithm: chunked parallel delta rule with chunk L=128.
#   Within a chunk: (I + A) U = V - K @ S0, A = tril(K K^T, -1).
#   (I+A)^-1 = (I+F1)(I+F2)(I+F3)(I+F4) with commuting factors built from
#   powers of B = -A (base-4 digits):
#     F1 = B+B^2+B^3, F2 = B^4+B^8+B^12, F3 = B^16+B^32+B^48, F4 = B^64.
#   Powers are maintained as (upper, lower) transpose pairs so every matmul
#   has its lhsT available without explicit transposes.
#
# Output path uses associativity to avoid materializing q:
#   y Wo^T = q @ kv @ Wo^T = x @ (Wgq^T kv Wo^T) + bgq @ (kv Wo^T)
# so the T-sized projections are only x->K, x->V, x->out.
#
# Layout strategy: a few large cast-DMA loads (f32->bf16), all transposes
# done on the TensorEngine via identity matmuls (no DMA transposes). The
# power-chain PSUM pools are declared first so they land on banks released
# early, letting each chunk's (state-independent) inverse factors compute
# during the projection phase; the serial solve then streams with the
# factors already in SBUF.

import numpy as np

N, T, C = 8, 2048, 512
L = 128
NCH = T // L  # 16 chunks
CT = C // 128  # 4 c-tiles

_CACHE = {}


def _build():
    import concourse.bacc as bacc
    import concourse.mybir as mybir
    import concourse.tile as tile
    from concourse.bass import ts, ds
    from concourse.masks import (
        make_identity,
        make_lower_triangular,
        make_upper_triangular,
    )

    f32 = mybir.dt.float32
    f32r = mybir.dt.float32r
    bf16 = mybir.dt.bfloat16
    AF = mybir.ActivationFunctionType

    nc = bacc.Bacc("TRN2")
    x = nc.declare_dram_parameter("x", [T, C], f32, isOutput=False)
    Wg = nc.declare_dram_parameter("Wg", [3 * C, C], f32, isOutput=False)
    bg = nc.declare_dram_parameter("bg", [3 * C], f32, isOutput=False)
    Wlr = nc.declare_dram_parameter("Wlr", [1, C], f32, isOutput=False)
    blr = nc.declare_dram_parameter("blr", [1], f32, isOutput=False)
    Wo = nc.declare_dram_parameter("Wo", [C, C], f32, isOutput=False)
    bo = nc.declare_dram_parameter("bo", [C], f32, isOutput=False)
    out = nc.declare_dram_parameter("out", [T, C], f32, isOutput=True)

    mm = nc.tensor.matmul

    with tile.TileContext(nc) as tc:
        with tc.tile_pool(name="persist", bufs=1) as P:
            # ---- constants / small tensors ----
            maskU = P.tile([128, 128], f32, name="maskU")
            maskL = P.tile([128, 128], f32, name="maskL")
            ones_bf = P.tile([1, 128], bf16, name="ones_bf")
            nc.vector.memset(ones_bf[:], 1.0)
            ident_bf = P.tile([128, 128], bf16, name="ident_bf")
            maskU2 = P.tile([128, 256], f32, name="maskU2")
            maskL2 = P.tile([128, 256], f32, name="maskL2")
            ident_f32 = P.tile([128, 128], f32, name="ident_f32")
            ident_fr = P.tile([128, 128], f32r, name="ident_fr")

            # ---- persistent tensors ----
            xT = [P.tile([128, T], bf16, name=f"xT{i}") for i in range(CT)]
            WgTk = [P.tile([128, C], bf16, name=f"WgTk{i}") for i in range(CT)]
            WgTv = [P.tile([128, C], bf16, name=f"WgTv{i}") for i in range(CT)]
            WoT = [P.tile([128, C], bf16, name=f"WoT{i}") for i in range(CT)]
            wgq_sb = P.tile([128, 4 * C], bf16, name="wgq_sb")
            Khat = [P.tile([128, C], f32r, name=f"Khat{i}") for i in range(NCH)]
            # chunk-major K^T: KTc[i][:, ci*128:(ci+1)*128] = Khat_i^T c-block
            KTc = [P.tile([128, C], bf16, name=f"KTc{i}") for i in range(NCH)]
            U = [P.tile([128, C], f32r, name=f"U{i}") for i in range(NCH)]
            # Sn = -S (negated state, bf16 accumulation) so the K@S term
            # accumulates positively in the solve
            S_bf = [P.tile([128, C], bf16, name=f"Sb{i}") for i in range(CT)]
            KVT = [P.tile([128, C], bf16, name=f"KVT{i}") for i in range(CT)]
            KVW = [P.tile([128, C], bf16, name=f"KVW{i}") for i in range(CT)]
            Mw = [P.tile([128, C], bf16, name=f"Mw{i}") for i in range(CT)]
            rb = P.tile([1, C], f32, name="rb")
            rb_bc = P.tile([128, C], f32, name="rb_bc")
            blr_bc = P.tile([128, 1], f32, name="blr_bc")
            lrn = P.tile([128, NCH], f32, name="lrn")
            KVf = [P.tile([128, C], f32, name=f"KVf{i}") for i in range(CT)]

            # ============ phase A: loads + PE transposes + K projections ====
            with tc.tile_pool(name="stage", bufs=1) as STG, \
                 tc.tile_pool(name="sbB", bufs=3) as SBB, \
                 tc.tile_pool(name="small", bufs=6) as SMALL, \
                 tc.tile_pool(name="psTR", bufs=3, space="PSUM") as TRP, \
                 tc.tile_pool(name="psTK", bufs=1, space="PSUM") as TRK, \
                 tc.tile_pool(name="psB", bufs=3, space="PSUM") as PSB, \
                 tc.tile_pool(name="psLr", bufs=1, space="PSUM") as PSLR:
                # large cast loads, rearranged so row-chunk j lands in
                # column block j: sb[p, j*512 + c] = src[j*128 + p, c]
                def big_load(dst, src_rows):
                    nc.gpsimd.dma_start(
                        out=dst[:].rearrange("p (j c) -> p j c", j=4),
                        in_=src_rows.rearrange("(j p) c -> p j c", p=128),
                    )

                x_sb = [STG.tile([128, 4 * C], bf16, name=f"x_sb{g}")
                        for g in range(4)]
                wgk_sb = STG.tile([128, 4 * C], bf16, name="wgk_sb")
                big_load(wgk_sb, Wg[C:2 * C, :])
                make_identity(nc, ident_bf[:])
                big_load(x_sb[0], x[0:512, :])
                # small bias loads early: bgk gates the first kproj's stop,
                # bgv the first V-proj; behind all big loads they'd land ~16us
                bgk_sb = P.tile([1, C], bf16, name="bgk_sb")
                nc.gpsimd.dma_start(out=bgk_sb[:], in_=bg[C:2 * C])
                bgv_sb = P.tile([1, C], bf16, name="bgv_sb")
                nc.gpsimd.dma_start(out=bgv_sb[:], in_=bg[2 * C:3 * C])
                wgv_sb = STG.tile([128, 4 * C], bf16, name="wgv_sb")
                big_load(wgv_sb, Wg[2 * C:3 * C, :])
                for g in range(1, 4):
                    big_load(x_sb[g], x[g * 512:(g + 1) * 512, :])
                WlrT = P.tile([128, CT], bf16, name="WlrT")
                nc.gpsimd.dma_start(
                    out=WlrT[:], in_=Wlr[0, :].rearrange("(i p) -> p i", p=128)
                )
                nc.gpsimd.dma_start(
                    out=blr_bc[:],
                    in_=blr[:].rearrange("(o c) -> o c", o=1)
                    .to_broadcast((128, 1)),
                )
                big_load(wgq_sb, Wg[0:C, :])
                wo_sb = STG.tile([128, 4 * C], bf16, name="wo_sb")
                big_load(wo_sb, Wo[:, :])
                bo_sb = P.tile([1, C], bf16, name="bo_sb")
                nc.gpsimd.dma_start(out=bo_sb[:], in_=bo[:])
                bgqT = P.tile([128, CT], bf16, name="bgqT")
                nc.gpsimd.dma_start(
                    out=bgqT[:], in_=bg[0:C].rearrange("(i p) -> p i", p=128)
                )

                # masks / remaining identities (needed from ~25us on)
                make_upper_triangular(nc, maskU[:], val=-1.0, diag=False)
                make_lower_triangular(nc, maskL[:], val=-1.0, diag=False)
                nc.vector.tensor_copy(maskU2[:, 0:128], maskU[:])
                nc.vector.tensor_copy(maskU2[:, 128:256], maskU[:])
                nc.vector.tensor_copy(maskL2[:, 0:128], maskL[:])
                nc.vector.tensor_copy(maskL2[:, 128:256], maskL[:])
                make_identity(nc, ident_f32[:])
                nc.vector.tensor_copy(ident_fr[:], ident_f32[:])

                # PE transposes: [128,128] tiles via identity matmul.
                def transpose_tiles(src, ci):
                    tp = TRP.tile([128, 4 * 128], bf16, name="tp")
                    for j in range(4):
                        nc.tensor.transpose(
                            tp[:, ts(j, 128)],
                            src[:, ds(j * 512 + ci * 128, 128)],
                            ident_bf[:],
                        )
                    return tp

                for ci in range(CT):
                    tp = transpose_tiles(wgk_sb, ci)
                    nc.any.tensor_copy(WgTk[ci][:], tp[:])
                for ci in range(CT):
                    tp = transpose_tiles(x_sb[0], ci)
                    nc.any.tensor_copy(xT[ci][:, ds(0, 512)], tp[:])
                for ci in range(CT):
                    tp = transpose_tiles(wgv_sb, ci)
                    nc.any.tensor_copy(WgTv[ci][:], tp[:])
                for g in range(1, 4):
                    for ci in range(CT):
                        tp = transpose_tiles(x_sb[g], ci)
                        nc.any.tensor_copy(xT[ci][:, ds(g * 512, 512)], tp[:])
                for ci in range(CT):
                    tp = transpose_tiles(wo_sb, ci)
                    nc.any.tensor_copy(WoT[ci][:], tp[:])

                # K projection + normalization; K^T per chunk (bf16)
                for tj in range(NCH):
                    kps = PSB.tile([128, C], f32, name="kps")
                    for ci in range(CT):
                        mm(kps[:], lhsT=xT[ci][:, ts(tj, 128)], rhs=WgTk[ci][:],
                           start=(ci == 0), stop=False)
                    mm(kps[:], lhsT=ones_bf[:], rhs=bgk_sb[:], start=False,
                       stop=True)
                    sq = SBB.tile([128, C], f32, name="sq")
                    n2 = SMALL.tile([128, 1], f32, name="n2")
                    nc.scalar.activation(sq[:], kps[:], AF.Square, accum_out=n2[:])
                    nrm = SMALL.tile([128, 1], f32, name="nrm")
                    nc.scalar.sqrt(nrm[:], n2[:])
                    rn = SMALL.tile([128, 1], f32, name="rn")
                    nc.vector.reciprocal(rn[:], nrm[:])
                    nc.vector.tensor_scalar_mul(Khat[tj][:], kps[:], rn[:])
                    ktp = TRK.tile([128, C], f32r, name="ktp")
                    for ci in range(CT):
                        nc.tensor.transpose(
                            ktp[:, ts(ci, 128)],
                            Khat[tj][:, ts(ci, 128)],
                            ident_fr[:],
                        )
                    nc.any.tensor_copy(KTc[tj][:], ktp[:])

                # lr column per chunk: lrn[:, i] = x_chunk @ Wlr^T + blr
                lps = PSLR.tile([128, NCH], f32, name="lps")
                for i in range(NCH):
                    for ci in range(CT):
                        mm(lps[:, i:i + 1], lhsT=xT[ci][:, ts(i, 128)],
                           rhs=WlrT[:, ci:ci + 1], start=(ci == 0),
                           stop=(ci == 3))
                nc.any.tensor_scalar_add(lrn[:], lps[:], blr_bc[:])

            # ================= phase C: delta-rule recurrence ===============
            # psPa/psPb are declared first so they are assigned the banks
            # released earliest by phase A, letting the per-chunk inverse
            # factors (independent of the recurrence state) run ahead.
            with tc.tile_pool(name="sbP", bufs=3) as SBP, \
                 tc.tile_pool(name="sbF", bufs=3) as SBF, \
                 tc.tile_pool(name="sbM", bufs=NCH // 2) as SBM, \
                 tc.tile_pool(name="sbG", bufs=18) as SBG, \
                 tc.tile_pool(name="sbU", bufs=3) as SBU, \
                 tc.tile_pool(name="sbLR", bufs=5) as SBLR, \
                 tc.tile_pool(name="psPa", bufs=1, space="PSUM") as PSPA, \
                 tc.tile_pool(name="psPb", bufs=1, space="PSUM") as PSPB, \
                 tc.tile_pool(name="psU", bufs=4, space="PSUM") as PSU, \
                 tc.tile_pool(name="psSD", bufs=2, space="PSUM") as PSSD:
                for ci in range(CT):
                    nc.gpsimd.memset(S_bf[ci][:], 0.0)
                for vi in range(CT):
                    nc.gpsimd.memset(KVf[vi][:], 0.0)

                gxns = {}
                lrus = {}
                Mps = {}
                for i in range(NCH):
                    if i % 2 == 0:
                        # ---- lockstep inverse-factor chain for the chunk
                        # pair (a, b) = (i, i+1). Region layout of every
                        # level tile: {X_a | X_b | Xl_a | Xl_b} so each
                        # level is ONE [128,512] bank with ONE evacuation.
                        a, b = i, i + 1
                        PSP = PSPA if (i // 2) % 2 == 0 else PSPB
                        gps = PSP.tile([128, 256], f32, name="pp")
                        for ci in range(CT):
                            ka = KTc[a][:, ts(ci, 128)]
                            mm(gps[:, 0:128], lhsT=ka, rhs=ka,
                               start=(ci == 0), stop=(ci == 3))
                        for ci in range(CT):
                            kb = KTc[b][:, ts(ci, 128)]
                            mm(gps[:, 128:256], lhsT=kb, rhs=kb,
                               start=(ci == 0), stop=(ci == 3))
                        BuP = SBP.tile([128, 256], bf16, name="BuP")
                        BlP = SBP.tile([128, 256], bf16, name="BlP")
                        nc.any.tensor_mul(BuP[:], gps[:], maskU2[:])
                        nc.any.tensor_mul(BlP[:], gps[:], maskL2[:])

                        def pair2(prev_u, prev_l, name):
                            # squares both chunks: out_u = l^T u, out_l = u^T l
                            ps = PSP.tile([128, 512], f32, name="pp")
                            for s in range(2):
                                pu = prev_u[:, ts(s, 128)]
                                pl = prev_l[:, ts(s, 128)]
                                mm(ps[:, ds(s * 128, 128)], lhsT=pl, rhs=pu,
                                   start=True, stop=True)
                                mm(ps[:, ds(256 + s * 128, 128)], lhsT=pu,
                                   rhs=pl, start=True, stop=True)
                            t = SBP.tile([128, 512], bf16, name=name)
                            if (i // 2) % 2 == 0:
                                nc.vector.tensor_copy(t[:], ps[:])
                            else:
                                nc.scalar.activation(t[:], ps[:], AF.Identity)
                            return t[:, 0:256], t[:, 256:512]

                        B2uP, B2lP = pair2(BuP, BlP, "B22")
                        B4uP, B4lP = pair2(B2uP, B2lP, "B44")
                        B8uP, B8lP = pair2(B4uP, B4lP, "B88")
                        B16uP, B16lP = pair2(B8uP, B8lP, "B1616")
                        B32uP, B32lP = pair2(B16uP, B16lP, "B3232")

                        # fp1X: {B3u_a | B3u_b | B48u_a | B48u_b}
                        fp1 = PSP.tile([128, 512], f32, name="pp")
                        for s in range(2):
                            mm(fp1[:, ds(s * 128, 128)],
                               lhsT=BlP[:, ts(s, 128)], rhs=BuP[:, ts(s, 128)],
                               start=True, stop=False)
                            mm(fp1[:, ds(s * 128, 128)],
                               lhsT=B2lP[:, ts(s, 128)], rhs=BuP[:, ts(s, 128)],
                               start=False, stop=True)
                            mm(fp1[:, ds(256 + s * 128, 128)],
                               lhsT=B8lP[:, ts(s, 128)], rhs=B8uP[:, ts(s, 128)],
                               start=True, stop=False)
                            mm(fp1[:, ds(256 + s * 128, 128)],
                               lhsT=B16lP[:, ts(s, 128)], rhs=B32uP[:, ts(s, 128)],
                               start=False, stop=True)
                        F1P = SBF.tile([128, 256], bf16, name="F1P")
                        nc.any.tensor_add(F1P[:], fp1[:, 0:256], BuP[:])
                        F3P = SBF.tile([128, 256], bf16, name="F3P")
                        nc.any.tensor_add(F3P[:], fp1[:, 256:512], B32uP)

                        # fp2X: {B12u_a | B12u_b | B12l_a | B12l_b}
                        fp2 = PSP.tile([128, 512], f32, name="pp")
                        for s in range(2):
                            mm(fp2[:, ds(s * 128, 128)],
                               lhsT=B2lP[:, ts(s, 128)], rhs=B2uP[:, ts(s, 128)],
                               start=True, stop=False)
                            mm(fp2[:, ds(s * 128, 128)],
                               lhsT=B4lP[:, ts(s, 128)], rhs=B8uP[:, ts(s, 128)],
                               start=False, stop=True)
                            mm(fp2[:, ds(256 + s * 128, 128)],
                               lhsT=B2uP[:, ts(s, 128)], rhs=B2lP[:, ts(s, 128)],
                               start=True, stop=False)
                            mm(fp2[:, ds(256 + s * 128, 128)],
                               lhsT=B8uP[:, ts(s, 128)], rhs=B4lP[:, ts(s, 128)],
                               start=False, stop=True)
                        F2P = SBF.tile([128, 256], bf16, name="F2P")
                        nc.any.tensor_add(F2P[:], fp2[:, 0:256], B8uP)
                        F2lP = SBF.tile([128, 256], bf16, name="F2lP")
                        nc.any.tensor_add(F2lP[:], fp2[:, 256:512], B8lP)

                        # ps64X: {B64u_a | B64u_b | B64l_a | B64l_b}
                        ps64 = PSP.tile([128, 512], f32, name="pp")
                        for s in range(2):
                            mm(ps64[:, ds(s * 128, 128)],
                               lhsT=B32lP[:, ts(s, 128)], rhs=B32uP[:, ts(s, 128)],
                               start=True, stop=True)
                            mm(ps64[:, ds(256 + s * 128, 128)],
                               lhsT=B32uP[:, ts(s, 128)], rhs=B32lP[:, ts(s, 128)],
                               start=True, stop=True)
                        F44P = SBF.tile([128, 512], bf16, name="F44P")
                        if (i // 2) % 2 == 0:
                            nc.scalar.activation(F44P[:], ps64[:], AF.Identity)
                        else:
                            nc.vector.tensor_copy(F44P[:], ps64[:])
                        F4uP, F4lP = F44P[:, 0:256], F44P[:, 256:512]

                        # mpsX: {M12u_a | M12u_b | M34u_a | M34u_b}
                        mps = PSP.tile([128, 512], f32, name="pp")
                        for s in range(2):
                            mm(mps[:, ds(s * 128, 128)],
                               lhsT=F2lP[:, ts(s, 128)], rhs=F1P[:, ts(s, 128)],
                               start=True, stop=False)
                            mm(mps[:, ds(s * 128, 128)],
                               lhsT=ident_bf[:], rhs=F1P[:, ts(s, 128)],
                               start=False, stop=False)
                            mm(mps[:, ds(s * 128, 128)],
                               lhsT=ident_bf[:], rhs=F2P[:, ts(s, 128)],
                               start=False, stop=True)
                            mm(mps[:, ds(256 + s * 128, 128)],
                               lhsT=F4lP[:, ts(s, 128)], rhs=F3P[:, ts(s, 128)],
                               start=True, stop=False)
                            mm(mps[:, ds(256 + s * 128, 128)],
                               lhsT=ident_bf[:], rhs=F3P[:, ts(s, 128)],
                               start=False, stop=False)
                            mm(mps[:, ds(256 + s * 128, 128)],
                               lhsT=ident_bf[:], rhs=F4uP[:, ts(s, 128)],
                               start=False, stop=True)
                        MpP = SBM.tile([128, 512], bf16, name="Mp")
                        nc.any.tensor_copy(MpP[:], mps[:])
                        Mps[i // 2] = MpP

                    # cross-chunk correction matrices within the quad
                    # (independent of U/S; can run ahead):
                    # gxns[src] with gxn^T @ U_src = -K_i K_src^T U_src
                    for src in range(i - i % 2, i):
                        gx = (PSPA if (i // 2) % 2 == 0 else PSPB).tile(
                            [128, 256], f32, name="pp")
                        for ci in range(CT):
                            mm(gx[:, 0:128], lhsT=KTc[src][:, ts(ci, 128)],
                               rhs=KTc[i][:, ts(ci, 128)],
                               start=(ci == 0), stop=(ci == 3))
                        g_t = SBG.tile([128, 128], f32r, name="gxn")
                        nc.vector.tensor_scalar_mul(g_t[:], gx[:, 0:128], -1.0)
                        gxns[(i, src)] = g_t

                    # --- U_i = (I+F1)(I+F2)(I+F3)(I+F4) (V_i - Khat_i S0) ---
                    # Chunks are paired: odd chunk i reads the pair-start state
                    # S0 plus an explicit cross term -A21 @ U_{i-1}.
                    ups = PSU.tile([128, C], f32, name="ups")
                    for ci in range(CT):
                        mm(ups[:], lhsT=xT[ci][:, ts(i, 128)],
                           rhs=WgTv[ci][:],
                           start=(ci == 0), stop=False)
                    mm(ups[:], lhsT=ones_bf[:], rhs=bgv_sb[:],
                       start=False, stop=(i <= 1))
                    if i >= 2:
                        for ci in range(CT):
                            mm(ups[:], lhsT=KTc[i][:, ts(ci, 128)],
                               rhs=S_bf[ci][:],
                               start=False, stop=(ci == 3))
                    for src in range(i - i % 2, i):
                        mm(ups[:], lhsT=gxns[(i, src)][:], rhs=U[src][:],
                           start=False, stop=True,
                           skip_group_check=True)
                    MpP = Mps[i // 2]
                    M12u = MpP[:, ds((i % 2) * 128, 128)]
                    M34u = MpP[:, ds(256 + (i % 2) * 128, 128)]
                    ucur = SBU.tile([128, C], bf16, name="ua")
                    nc.any.tensor_copy(ucur[:], ups[:])  # U_0 = V - Khat S0
                    mm(ups[:], lhsT=M34u, rhs=ucur[:], start=False, stop=True,
                       skip_group_check=True)
                    umid = SBU.tile([128, C], bf16, name="ub")
                    nc.any.tensor_copy(umid[:], ups[:])
                    mm(ups[:], lhsT=M12u, rhs=umid[:], start=False, stop=True,
                       skip_group_check=True)
                    nc.any.tensor_copy(U[i][:], ups[:])
                    lru = SBLR.tile([128, C], f32r, name="lru")
                    nc.vector.tensor_scalar_mul(lru[:], U[i][:], lrn[:, i:i + 1])
                    lrus[i] = lru

                    # --- Sn -= Khat^T U for the quad, at quad end (bf16) ---
                    if i % 2 == 1 and i < NCH - 1:
                        for ci in range(CT):
                            sd = PSSD.tile([128, C], f32, name="sd")
                            for j in range(2):
                                mm(sd[:], lhsT=Khat[i - 1 + j][:, ts(ci, 128)],
                                   rhs=U[i - 1 + j][:], start=(j == 0),
                                   stop=(j == 1))
                            nc.vector.tensor_sub(S_bf[ci][:], S_bf[ci][:], sd[:])
                    # --- kv partial: KVf[vi] += sum_j lru_j^T(vi) Khat_j ---
                    # normally per quad; the LAST quad is split per pair so
                    # only chunks 14,15 remain on the tail after U_15.
                    kv_span = None
                    if i == NCH - 2:
                        kv_span = (i - 2, 2, False)
                    elif i == NCH - 1:
                        kv_span = (i - 1, 2, True)
                    elif i % 4 == 3 and i < NCH - 4:
                        kv_span = (i - 3, 4, False)
                    if kv_span is not None:
                        j0, nj, last = kv_span
                        for vi in range(CT):
                            kvp = PSSD.tile([128, C], f32, name="sd")
                            for j in range(nj):
                                cj = j0 + j
                                mm(kvp[:], lhsT=lrus[cj][:, ts(vi, 128)],
                                   rhs=Khat[cj][:], start=(j == 0),
                                   stop=(j == nj - 1))
                            if last:
                                # final partial: write the bf16 KVT directly
                                nc.vector.tensor_add(KVT[vi][:], kvp[:],
                                                     KVf[vi][:])
                            else:
                                nc.vector.tensor_add(KVf[vi][:], kvp[:],
                                                     KVf[vi][:])
                        if i % 4 == 3:
                            lrus.clear()

            # ================= phase D: outputs =============================
            with tc.tile_pool(name="sbD", bufs=4) as SBD, \
                 tc.tile_pool(name="sbZ", bufs=3) as SBZ, \
                 tc.tile_pool(name="psD", bufs=4, space="PSUM") as PSD:
                for ki in range(CT):
                    kwp = PSD.tile([128, C], f32, name="dps")
                    for vi in range(CT):
                        mm(kwp[:], lhsT=KVT[vi][:, ts(ki, 128)], rhs=WoT[vi][:],
                           start=(vi == 0), stop=(vi == 3))
                    nc.any.tensor_copy(KVW[ki][:], kwp[:])

                # Mw[m] = (Wgq^T @ KVW) block m; rb = bgq @ KVW + bo
                for m in range(CT):
                    mps2 = PSD.tile([128, C], f32, name="dps")
                    for kk in range(CT):
                        mm(mps2[:], lhsT=wgq_sb[:, ds(kk * 512 + m * 128, 128)],
                           rhs=KVW[kk][:], start=(kk == 0), stop=(kk == 3))
                    nc.any.tensor_copy(Mw[m][:], mps2[:])
                rps = PSD.tile([1, C], f32, name="dps")
                for ki in range(CT):
                    mm(rps[:], lhsT=bgqT[:, ki:ki + 1], rhs=KVW[ki][:],
                       start=(ki == 0), stop=False)
                mm(rps[:], lhsT=ones_bf[:, 0:1], rhs=bo_sb[:],
                   start=False, stop=True)
                nc.any.tensor_copy(rb[:], rps[:])
                nc.gpsimd.partition_broadcast(rb_bc[:], rb[:])

                # out chunks: z = x @ Mw + rb, stored 2 chunks per DMA
                for g in range(8):
                    zt2 = SBZ.tile([128, 2 * C], f32, name="zt2")
                    for j in range(2):
                        tj = g * 2 + j
                        zps = PSD.tile([128, C], f32, name="dps")
                        for ci in range(CT):
                            mm(zps[:], lhsT=xT[ci][:, ts(tj, 128)], rhs=Mw[ci][:],
                               start=(ci == 0), stop=(ci == 3))
                        nc.any.tensor_add(zt2[:, ds(j * 512, 512)], zps[:],
                                          rb_bc[:])
                    nc.sync.dma_start(
                        out=out[g * 256:(g + 1) * 256, :].rearrange(
                            "(j p) c -> p j c", p=128),
                        in_=zt2[:].rearrange("p (j c) -> p j c", j=2),
                    )

    nc.finalize()
    return nc


def _get_nc():
    if "nc" not in _CACHE:
        _CACHE["nc"] = _build()
    return _CACHE["nc"]


def _in_maps(inputs):
    def f(a):
        return np.ascontiguousarray(np.asarray(a, dtype=np.float32))

    x = f(inputs["x"])
    shared = {k: f(inputs[k]) for k in ("Wg", "bg", "Wlr", "blr", "Wo", "bo")}
    return [{"x": x[i], **shared} for i in range(N)]


def _run(in_maps, **kw):
    from concourse.bass_utils import run_bass_kernel_spmd

    nc = _get_nc()
    return run_bass_kernel_spmd(nc, in_maps, list(range(N)), **kw)


def kernel(**inputs) -> np.ndarray:
    res = _run(_in_maps(inputs))
    return np.stack([res.results[i]["out"] for i in range(N)]).astype(np.float32)



# revision 72
# speedup vs baseline: 1.0625x; 1.0188x over previous
# Bass/Trainium2 kernel for nn_Delta (DeltaNet-style recurrence).
#
# Problem (hardcoded): N=8, T=2048, C=512, fp32 I/O.
#   g = x @ Wg.T + bg ; q,k,v = split(g) ; lr = x @ Wlr.T + blr
#   k = k / ||k||
#   delta-rule scan:  u_t = v_t - k_t @ S ; S += outer(k_t, u_t)
#   kv = sum_t k_t (x) (lr_t * u_t) ; y = q @ kv ; out = y @ Wo.T + bo
#
# Sharding: data-parallel over N across the 8 cores (sample i -> core i),
# weights replicated. No collectives.
#
# Per-core algorithm: chunked parallel delta rule with chunk L=128.
#   Within a chunk: (I + A) U = V - K @ S0, A = tril(K K^T, -1).
#   (I+A)^-1 = (I+F1)(I+F2)(I+F3)(I+F4) with commuting factors built from
#   powers of B = -A (base-4 digits):
#     F1 = B+B^2+B^3, F2 = B^4+B^8+B^12, F3 = B^16+B^32+B^48, F4 = B^64.
#   Powers are maintained as (upper, lower) transpose pairs so every matmul
#   has its lhsT available without explicit transposes.
#
# Output path uses associativity to avoid materializing q:
#   y Wo^T = q @ kv @ Wo^T = x @ (Wgq^T kv Wo^T) + bgq @ (kv Wo^T)
# so the T-sized projections are only x->K, x->V, x->out.
#
# Layout strategy: a few large cast-DMA loads (f32->bf16), all transposes
# done on the TensorEngine via identity matmuls (no DMA transposes). The
# power-chain PSUM pools are declared first so they land on banks released
# early, letting each chunk's (state-independent) inverse factors compute
# during the projection phase; the serial solve then streams with the
# factors already in SBUF.

import numpy as np

N, T, C = 8, 2048, 512
L = 128
NCH = T // L  # 16 chunks
CT = C // 128  # 4 c-tiles

_CACHE = {}


def _build():
    import concourse.bacc as bacc
    import concourse.mybir as mybir
    import concourse.tile as tile
    from concourse.bass import ts, ds
    from concourse.masks import (
        make_identity,
        make_lower_triangular,
        make_upper_triangular,
    )

    f32 = mybir.dt.float32
    f32r = mybir.dt.float32r
    bf16 = mybir.dt.bfloat16
    AF = mybir.ActivationFunctionType

    nc = bacc.Bacc("TRN2")
    x = nc.declare_dram_parameter("x", [T, C], f32, isOutput=False)
    Wg = nc.declare_dram_parameter("Wg", [3 * C, C], f32, isOutput=False)
    bg = nc.declare_dram_parameter("bg", [3 * C], f32, isOutput=False)
    Wlr = nc.declare_dram_parameter("Wlr", [1, C], f32, isOutput=False)
    blr = nc.declare_dram_parameter("blr", [1], f32, isOutput=False)
    Wo = nc.declare_dram_parameter("Wo", [C, C], f32, isOutput=False)
    bo = nc.declare_dram_parameter("bo", [C], f32, isOutput=False)
    out = nc.declare_dram_parameter("out", [T, C], f32, isOutput=True)

    mm = nc.tensor.matmul

    with tile.TileContext(nc) as tc:
        with tc.tile_pool(name="persist", bufs=1) as P:
            # ---- constants / small tensors ----
            maskU = P.tile([128, 128], f32, name="maskU")
            maskL = P.tile([128, 128], f32, name="maskL")
            ones_bf = P.tile([1, 128], bf16, name="ones_bf")
            nc.vector.memset(ones_bf[:], 1.0)
            ident_bf = P.tile([128, 128], bf16, name="ident_bf")
            maskU2 = P.tile([128, 256], f32, name="maskU2")
            maskL2 = P.tile([128, 256], f32, name="maskL2")
            ident_f32 = P.tile([128, 128], f32, name="ident_f32")
            ident_fr = P.tile([128, 128], f32r, name="ident_fr")

            # ---- persistent tensors ----
            xT = [P.tile([128, T], bf16, name=f"xT{i}") for i in range(CT)]
            WgTk = [P.tile([128, C], bf16, name=f"WgTk{i}") for i in range(CT)]
            WgTv = [P.tile([128, C], bf16, name=f"WgTv{i}") for i in range(CT)]
            WoT = [P.tile([128, C], bf16, name=f"WoT{i}") for i in range(CT)]
            wgq_sb = P.tile([128, 4 * C], bf16, name="wgq_sb")
            Khat = [P.tile([128, C], f32r, name=f"Khat{i}") for i in range(NCH)]
            # chunk-major K^T: KTc[i][:, ci*128:(ci+1)*128] = Khat_i^T c-block
            KTc = [P.tile([128, C], bf16, name=f"KTc{i}") for i in range(NCH)]
            U = [P.tile([128, C], f32r, name=f"U{i}") for i in range(NCH)]
            # Sn = -S (negated state, bf16 accumulation) so the K@S term
            # accumulates positively in the solve
            S_bf = [P.tile([128, C], bf16, name=f"Sb{i}") for i in range(CT)]
            KVT = [P.tile([128, C], bf16, name=f"KVT{i}") for i in range(CT)]
            KVW = [P.tile([128, C], bf16, name=f"KVW{i}") for i in range(CT)]
            Mw = [P.tile([128, C], bf16, name=f"Mw{i}") for i in range(CT)]
            rb = P.tile([1, C], f32, name="rb")
            rb_bc = P.tile([128, C], f32, name="rb_bc")
            blr_bc = P.tile([128, 1], f32, name="blr_bc")
            lrn = P.tile([128, NCH], f32, name="lrn")
            KVf = [P.tile([128, C], f32, name=f"KVf{i}") for i in range(CT)]

            # ============ phase A: loads + PE transposes + K projections ====
            with tc.tile_pool(name="stage", bufs=1) as STG, \
                 tc.tile_pool(name="sbB", bufs=3) as SBB, \
                 tc.tile_pool(name="small", bufs=6) as SMALL, \
                 tc.tile_pool(name="psTR", bufs=3, space="PSUM") as TRP, \
                 tc.tile_pool(name="psTK", bufs=1, space="PSUM") as TRK, \
                 tc.tile_pool(name="psB", bufs=3, space="PSUM") as PSB, \
                 tc.tile_pool(name="psLr", bufs=1, space="PSUM") as PSLR:
                # large cast loads, rearranged so row-chunk j lands in
                # column block j: sb[p, j*512 + c] = src[j*128 + p, c]
                def big_load(dst, src_rows):
                    nc.gpsimd.dma_start(
                        out=dst[:].rearrange("p (j c) -> p j c", j=4),
                        in_=src_rows.rearrange("(j p) c -> p j c", p=128),
                    )

                x_sb = [STG.tile([128, 4 * C], bf16, name=f"x_sb{g}")
                        for g in range(4)]
                wgk_sb = STG.tile([128, 4 * C], bf16, name="wgk_sb")
                big_load(wgk_sb, Wg[C:2 * C, :])
                make_identity(nc, ident_bf[:])
                big_load(x_sb[0], x[0:512, :])
                # small bias loads early: bgk gates the first kproj's stop,
                # bgv the first V-proj; behind all big loads they'd land ~16us
                bgk_sb = P.tile([1, C], bf16, name="bgk_sb")
                nc.gpsimd.dma_start(out=bgk_sb[:], in_=bg[C:2 * C])
                bgv_sb = P.tile([1, C], bf16, name="bgv_sb")
                nc.gpsimd.dma_start(out=bgv_sb[:], in_=bg[2 * C:3 * C])
                wgv_sb = STG.tile([128, 4 * C], bf16, name="wgv_sb")
                big_load(wgv_sb, Wg[2 * C:3 * C, :])
                for g in range(1, 4):
                    big_load(x_sb[g], x[g * 512:(g + 1) * 512, :])
                WlrT = P.tile([128, CT], bf16, name="WlrT")
                nc.gpsimd.dma_start(
                    out=WlrT[:], in_=Wlr[0, :].rearrange("(i p) -> p i", p=128)
                )
                nc.gpsimd.dma_start(
                    out=blr_bc[:],
                    in_=blr[:].rearrange("(o c) -> o c", o=1)
                    .to_broadcast((128, 1)),
                )
                big_load(wgq_sb, Wg[0:C, :])
                wo_sb = STG.tile([128, 4 * C], bf16, name="wo_sb")
                big_load(wo_sb, Wo[:, :])
                bo_sb = P.tile([1, C], bf16, name="bo_sb")
                nc.gpsimd.dma_start(out=bo_sb[:], in_=bo[:])
                bgqT = P.tile([128, CT], bf16, name="bgqT")
                nc.gpsimd.dma_start(
                    out=bgqT[:], in_=bg[0:C].rearrange("(i p) -> p i", p=128)
                )

                # masks / remaining identities (needed from ~25us on)
                make_upper_triangular(nc, maskU[:], val=-1.0, diag=False)
                make_lower_triangular(nc, maskL[:], val=-1.0, diag=False)
                nc.vector.tensor_copy(maskU2[:, 0:128], maskU[:])
                nc.vector.tensor_copy(maskU2[:, 128:256], maskU[:])
                nc.vector.tensor_copy(maskL2[:, 0:128], maskL[:])
                nc.vector.tensor_copy(maskL2[:, 128:256], maskL[:])
                make_identity(nc, ident_f32[:])
                nc.vector.tensor_copy(ident_fr[:], ident_f32[:])

                # PE transposes: [128,128] tiles via identity matmul.
                def transpose_tiles(src, ci):
                    tp = TRP.tile([128, 4 * 128], bf16, name="tp")
                    for j in range(4):
                        nc.tensor.transpose(
                            tp[:, ts(j, 128)],
                            src[:, ds(j * 512 + ci * 128, 128)],
                            ident_bf[:],
                        )
                    return tp

                for ci in range(CT):
                    tp = transpose_tiles(wgk_sb, ci)
                    nc.any.tensor_copy(WgTk[ci][:], tp[:])
                for ci in range(CT):
                    tp = transpose_tiles(x_sb[0], ci)
                    nc.any.tensor_copy(xT[ci][:, ds(0, 512)], tp[:])
                for ci in range(CT):
                    tp = transpose_tiles(wgv_sb, ci)
                    nc.any.tensor_copy(WgTv[ci][:], tp[:])
                for g in range(1, 4):
                    for ci in range(CT):
                        tp = transpose_tiles(x_sb[g], ci)
                        nc.any.tensor_copy(xT[ci][:, ds(g * 512, 512)], tp[:])
                for ci in range(CT):
                    tp = transpose_tiles(wo_sb, ci)
                    nc.any.tensor_copy(WoT[ci][:], tp[:])

                # K projection + normalization; K^T per chunk (bf16)
                for tj in range(NCH):
                    kps = PSB.tile([128, C], f32, name="kps")
                    for ci in range(CT):
                        mm(kps[:], lhsT=xT[ci][:, ts(tj, 128)], rhs=WgTk[ci][:],
                           start=(ci == 0), stop=False)
                    mm(kps[:], lhsT=ones_bf[:], rhs=bgk_sb[:], start=False,
                       stop=True)
                    sq = SBB.tile([128, C], f32, name="sq")
                    n2 = SMALL.tile([128, 1], f32, name="n2")
                    nc.scalar.activation(sq[:], kps[:], AF.Square, accum_out=n2[:])
                    nrm = SMALL.tile([128, 1], f32, name="nrm")
                    nc.scalar.sqrt(nrm[:], n2[:])
                    rn = SMALL.tile([128, 1], f32, name="rn")
                    nc.vector.reciprocal(rn[:], nrm[:])
                    nc.vector.tensor_scalar_mul(Khat[tj][:], kps[:], rn[:])
                    ktp = TRK.tile([128, C], f32r, name="ktp")
                    for ci in range(CT):
                        nc.tensor.transpose(
                            ktp[:, ts(ci, 128)],
                            Khat[tj][:, ts(ci, 128)],
                            ident_fr[:],
                        )
                    nc.any.tensor_copy(KTc[tj][:], ktp[:])

                # lr column per chunk: lrn[:, i] = x_chunk @ Wlr^T + blr
                lps = PSLR.tile([128, NCH], f32, name="lps")
                for i in range(NCH):
                    for ci in range(CT):
                        mm(lps[:, i:i + 1], lhsT=xT[ci][:, ts(i, 128)],
                           rhs=WlrT[:, ci:ci + 1], start=(ci == 0),
                           stop=(ci == 3))
                nc.any.tensor_scalar_add(lrn[:], lps[:], blr_bc[:])

            # ================= phase C: delta-rule recurrence ===============
            # psPa/psPb are declared first so they are assigned the banks
            # released earliest by phase A, letting the per-chunk inverse
            # factors (independent of the recurrence state) run ahead.
            with tc.tile_pool(name="sbP", bufs=3) as SBP, \
                 tc.tile_pool(name="sbF", bufs=3) as SBF, \
                 tc.tile_pool(name="sbM", bufs=NCH // 2) as SBM, \
                 tc.tile_pool(name="sbG", bufs=18) as SBG, \
                 tc.tile_pool(name="sbU", bufs=3) as SBU, \
                 tc.tile_pool(name="sbLR", bufs=5) as SBLR, \
                 tc.tile_pool(name="psPa", bufs=1, space="PSUM") as PSPA, \
                 tc.tile_pool(name="psPb", bufs=1, space="PSUM") as PSPB, \
                 tc.tile_pool(name="psU", bufs=4, space="PSUM") as PSU, \
                 tc.tile_pool(name="psSD", bufs=2, space="PSUM") as PSSD:
                for ci in range(CT):
                    nc.gpsimd.memset(S_bf[ci][:], 0.0)
                for vi in range(CT):
                    nc.gpsimd.memset(KVf[vi][:], 0.0)

                gxns = {}
                lrus = {}
                Mps = {}
                for i in range(NCH):
                    if i % 2 == 0:
                        # ---- lockstep inverse-factor chain for the chunk
                        # pair (a, b) = (i, i+1). Region layout of every
                        # level tile: {X_a | X_b | Xl_a | Xl_b} so each
                        # level is ONE [128,512] bank with ONE evacuation.
                        a, b = i, i + 1
                        PSP = PSPA if (i // 2) % 2 == 0 else PSPB
                        gps = PSP.tile([128, 256], f32, name="pp")
                        for ci in range(CT):
                            ka = KTc[a][:, ts(ci, 128)]
                            mm(gps[:, 0:128], lhsT=ka, rhs=ka,
                               start=(ci == 0), stop=(ci == 3))
                        for ci in range(CT):
                            kb = KTc[b][:, ts(ci, 128)]
                            mm(gps[:, 128:256], lhsT=kb, rhs=kb,
                               start=(ci == 0), stop=(ci == 3))
                        BuP = SBP.tile([128, 256], bf16, name="BuP")
                        BlP = SBP.tile([128, 256], bf16, name="BlP")
                        nc.any.tensor_mul(BuP[:], gps[:], maskU2[:])
                        nc.any.tensor_mul(BlP[:], gps[:], maskL2[:])

                        def pair2(prev_u, prev_l, name):
                            # squares both chunks: out_u = l^T u, out_l = u^T l
                            ps = PSP.tile([128, 512], f32, name="pp")
                            for s in range(2):
                                pu = prev_u[:, ts(s, 128)]
                                pl = prev_l[:, ts(s, 128)]
                                mm(ps[:, ds(s * 128, 128)], lhsT=pl, rhs=pu,
                                   start=True, stop=True)
                                mm(ps[:, ds(256 + s * 128, 128)], lhsT=pu,
                                   rhs=pl, start=True, stop=True)
                            t = SBP.tile([128, 512], bf16, name=name)
                            if (i // 2) % 2 == 0:
                                nc.vector.tensor_copy(t[:], ps[:])
                            else:
                                nc.scalar.activation(t[:], ps[:], AF.Identity)
                            return t[:, 0:256], t[:, 256:512]

                        B2uP, B2lP = pair2(BuP, BlP, "B22")
                        B4uP, B4lP = pair2(B2uP, B2lP, "B44")
                        B8uP, B8lP = pair2(B4uP, B4lP, "B88")
                        B16uP, B16lP = pair2(B8uP, B8lP, "B1616")
                        B32uP, B32lP = pair2(B16uP, B16lP, "B3232")

                        # fp1X: {B3u_a | B3u_b | B48u_a | B48u_b}
                        fp1 = PSP.tile([128, 512], f32, name="pp")
                        for s in range(2):
                            mm(fp1[:, ds(s * 128, 128)],
                               lhsT=BlP[:, ts(s, 128)], rhs=BuP[:, ts(s, 128)],
                               start=True, stop=False)
                            mm(fp1[:, ds(s * 128, 128)],
                               lhsT=B2lP[:, ts(s, 128)], rhs=BuP[:, ts(s, 128)],
                               start=False, stop=True)
                            mm(fp1[:, ds(256 + s * 128, 128)],
                               lhsT=B8lP[:, ts(s, 128)], rhs=B8uP[:, ts(s, 128)],
                               start=True, stop=False)
                            mm(fp1[:, ds(256 + s * 128, 128)],
                               lhsT=B16lP[:, ts(s, 128)], rhs=B32uP[:, ts(s, 128)],
                               start=False, stop=True)
                        F1P = SBF.tile([128, 256], bf16, name="F1P")
                        nc.any.tensor_add(F1P[:], fp1[:, 0:256], BuP[:])
                        F3P = SBF.tile([128, 256], bf16, name="F3P")
                        nc.any.tensor_add(F3P[:], fp1[:, 256:512], B32uP)

                        # fp2X: {B12u_a | B12u_b | B12l_a | B12l_b}
                        fp2 = PSP.tile([128, 512], f32, name="pp")
                        for s in range(2):
                            mm(fp2[:, ds(s * 128, 128)],
                               lhsT=B2lP[:, ts(s, 128)], rhs=B2uP[:, ts(s, 128)],
                               start=True, stop=False)
                            mm(fp2[:, ds(s * 128, 128)],
                               lhsT=B4lP[:, ts(s, 128)], rhs=B8uP[:, ts(s, 128)],
                               start=False, stop=True)
                            mm(fp2[:, ds(256 + s * 128, 128)],
                               lhsT=B2uP[:, ts(s, 128)], rhs=B2lP[:, ts(s, 128)],
                               start=True, stop=False)
                            mm(fp2[:, ds(256 + s * 128, 128)],
                               lhsT=B8uP[:, ts(s, 128)], rhs=B4lP[:, ts(s, 128)],
                               start=False, stop=True)
                        F2P = SBF.tile([128, 256], bf16, name="F2P")
                        nc.any.tensor_add(F2P[:], fp2[:, 0:256], B8uP)
                        F2lP = SBF.tile([128, 256], bf16, name="F2lP")
                        nc.any.tensor_add(F2lP[:], fp2[:, 256:512], B8lP)

                        # ps64X: {B64u_a | B64u_b | B64l_a | B64l_b}
                        ps64 = PSP.tile([128, 512], f32, name="pp")
                        for s in range(2):
                            mm(ps64[:, ds(s * 128, 128)],
                               lhsT=B32lP[:, ts(s, 128)], rhs=B32uP[:, ts(s, 128)],
                               start=True, stop=True)
                            mm(ps64[:, ds(256 + s * 128, 128)],
                               lhsT=B32uP[:, ts(s, 128)], rhs=B32lP[:, ts(s, 128)],
                               start=True, stop=True)
                        F44P = SBF.tile([128, 512], bf16, name="F44P")
                        if (i // 2) % 2 == 0:
                            nc.scalar.activation(F44P[:], ps64[:], AF.Identity)
                        else:
                            nc.vector.tensor_copy(F44P[:], ps64[:])
                        F4uP, F4lP = F44P[:, 0:256], F44P[:, 256:512]

                        # mpsX: {M12u_a | M12u_b | M34u_a | M34u_b}
                        mps = PSP.tile([128, 512], f32, name="pp")
                        for s in range(2):
                            mm(mps[:, ds(s * 128, 128)],
                               lhsT=F2lP[:, ts(s, 128)], rhs=F1P[:, ts(s, 128)],
                               start=True, stop=False)
                            mm(mps[:, ds(s * 128, 128)],
                               lhsT=ident_bf[:], rhs=F1P[:, ts(s, 128)],
                               start=False, stop=False)
                            mm(mps[:, ds(s * 128, 128)],
                               lhsT=ident_bf[:], rhs=F2P[:, ts(s, 128)],
                               start=False, stop=True)
                            mm(mps[:, ds(256 + s * 128, 128)],
                               lhsT=F4lP[:, ts(s, 128)], rhs=F3P[:, ts(s, 128)],
                               start=True, stop=False)
                            mm(mps[:, ds(256 + s * 128, 128)],
                               lhsT=ident_bf[:], rhs=F3P[:, ts(s, 128)],
                               start=False, stop=False)
                            mm(mps[:, ds(256 + s * 128, 128)],
                               lhsT=ident_bf[:], rhs=F4uP[:, ts(s, 128)],
                               start=False, stop=True)
                        MpP = SBM.tile([128, 512], bf16, name="Mp")
                        nc.any.tensor_copy(MpP[:], mps[:])
                        Mps[i // 2] = MpP

                    # cross-chunk correction matrices within the quad
                    # (independent of U/S; can run ahead):
                    # gxns[src] with gxn^T @ U_src = -K_i K_src^T U_src
                    for src in range(i - i % 2, i):
                        gx = (PSPA if (i // 2) % 2 == 0 else PSPB).tile(
                            [128, 256], f32, name="pp")
                        for ci in range(CT):
                            mm(gx[:, 0:128], lhsT=KTc[src][:, ts(ci, 128)],
                               rhs=KTc[i][:, ts(ci, 128)],
                               start=(ci == 0), stop=(ci == 3))
                        g_t = SBG.tile([128, 128], f32r, name="gxn")
                        nc.vector.tensor_scalar_mul(g_t[:], gx[:, 0:128], -1.0)
                        gxns[(i, src)] = g_t

                    # --- U_i = (I+F1)(I+F2)(I+F3)(I+F4) (V_i - Khat_i S0) ---
                    # Chunks are paired: odd chunk i reads the pair-start state
                    # S0 plus an explicit cross term -A21 @ U_{i-1}.
                    ups = PSU.tile([128, C], f32, name="ups")
                    for ci in range(CT):
                        mm(ups[:], lhsT=xT[ci][:, ts(i, 128)],
                           rhs=WgTv[ci][:],
                           start=(ci == 0), stop=False)
                    mm(ups[:], lhsT=ones_bf[:], rhs=bgv_sb[:],
                       start=False, stop=(i <= 1))
                    if i >= 2:
                        for ci in range(CT):
                            mm(ups[:], lhsT=KTc[i][:, ts(ci, 128)],
                               rhs=S_bf[ci][:],
                               start=False, stop=(ci == 3))
                    for src in range(i - i % 2, i):
                        mm(ups[:], lhsT=gxns[(i, src)][:], rhs=U[src][:],
                           start=False, stop=True,
                           skip_group_check=True)
                    MpP = Mps[i // 2]
                    M12u = MpP[:, ds((i % 2) * 128, 128)]
                    M34u = MpP[:, ds(256 + (i % 2) * 128, 128)]
                    ucur = SBU.tile([128, C], bf16, name="ua")
                    nc.any.tensor_copy(ucur[:], ups[:])  # U_0 = V - Khat S0
                    mm(ups[:], lhsT=M34u, rhs=ucur[:], start=False, stop=True,
                       skip_group_check=True)
                    umid = SBU.tile([128, C], bf16, name="ub")
                    nc.any.tensor_copy(umid[:], ups[:])
                    mm(ups[:], lhsT=M12u, rhs=umid[:], start=False, stop=True,
                       skip_group_check=True)
                    nc.any.tensor_copy(U[i][:], ups[:])
                    klr = SBLR.tile([128, C], f32r, name="lru")
                    nc.vector.tensor_scalar_mul(klr[:], Khat[i][:],
                                                lrn[:, i:i + 1])
                    lrus[i] = klr

                    # --- Sn -= Khat^T U for the quad, at quad end (bf16) ---
                    if i % 2 == 1 and i < NCH - 1:
                        for ci in range(CT):
                            sd = PSSD.tile([128, C], f32, name="sd")
                            for j in range(2):
                                mm(sd[:], lhsT=Khat[i - 1 + j][:, ts(ci, 128)],
                                   rhs=U[i - 1 + j][:], start=(j == 0),
                                   stop=(j == 1))
                            nc.vector.tensor_sub(S_bf[ci][:], S_bf[ci][:], sd[:])
                    # --- kv partial: KVf[vi] += sum_j lru_j^T(vi) Khat_j ---
                    # normally per quad; the LAST quad is split per pair so
                    # only chunks 14,15 remain on the tail after U_15.
                    kv_span = None
                    if i == NCH - 1:
                        kv_span = (i - 1, 2, True)
                    elif i % 2 == 0 and i >= 2:
                        kv_span = (i - 2, 2, False)
                    if kv_span is not None:
                        j0, nj, last = kv_span
                        for vi in range(CT):
                            kvp = PSSD.tile([128, C], f32, name="sd")
                            for j in range(nj):
                                cj = j0 + j
                                mm(kvp[:], lhsT=U[cj][:, ts(vi, 128)],
                                   rhs=lrus[cj][:], start=(j == 0),
                                   stop=(j == nj - 1))
                            if last:
                                # final partial: write the bf16 KVT directly
                                nc.vector.tensor_add(KVT[vi][:], kvp[:],
                                                     KVf[vi][:])
                            else:
                                nc.vector.tensor_add(KVf[vi][:], kvp[:],
                                                     KVf[vi][:])


            # ================= phase D: outputs =============================
            with tc.tile_pool(name="sbD", bufs=4) as SBD, \
                 tc.tile_pool(name="sbZ", bufs=3) as SBZ, \
                 tc.tile_pool(name="psD", bufs=4, space="PSUM") as PSD:
                for ki in range(CT):
                    kwp = PSD.tile([128, C], f32, name="dps")
                    for vi in range(CT):
                        mm(kwp[:], lhsT=KVT[vi][:, ts(ki, 128)], rhs=WoT[vi][:],
                           start=(vi == 0), stop=(vi == 3))
                    nc.any.tensor_copy(KVW[ki][:], kwp[:])

                # Mw[m] = (Wgq^T @ KVW) block m; rb = bgq @ KVW + bo
                for m in range(CT):
                    mps2 = PSD.tile([128, C], f32, name="dps")
                    for kk in range(CT):
                        mm(mps2[:], lhsT=wgq_sb[:, ds(kk * 512 + m * 128, 128)],
                           rhs=KVW[kk][:], start=(kk == 0), stop=(kk == 3))
                    nc.any.tensor_copy(Mw[m][:], mps2[:])
                rps = PSD.tile([1, C], f32, name="dps")
                for ki in range(CT):
                    mm(rps[:], lhsT=bgqT[:, ki:ki + 1], rhs=KVW[ki][:],
                       start=(ki == 0), stop=False)
                mm(rps[:], lhsT=ones_bf[:, 0:1], rhs=bo_sb[:],
                   start=False, stop=True)
                nc.any.tensor_copy(rb[:], rps[:])
                nc.gpsimd.partition_broadcast(rb_bc[:], rb[:])

                # out chunks: z = x @ Mw + rb, stored 2 chunks per DMA
                for g in range(8):
                    zt2 = SBZ.tile([128, 2 * C], f32, name="zt2")
                    for j in range(2):
                        tj = g * 2 + j
                        zps = PSD.tile([128, C], f32, name="dps")
                        for ci in range(CT):
                            mm(zps[:], lhsT=xT[ci][:, ts(tj, 128)], rhs=Mw[ci][:],
                               start=(ci == 0), stop=(ci == 3))
                        nc.any.tensor_add(zt2[:, ds(j * 512, 512)], zps[:],
                                          rb_bc[:])
                    nc.sync.dma_start(
                        out=out[g * 256:(g + 1) * 256, :].rearrange(
                            "(j p) c -> p j c", p=128),
                        in_=zt2[:].rearrange("p (j c) -> p j c", j=2),
                    )

    nc.finalize()
    return nc


def _get_nc():
    if "nc" not in _CACHE:
        _CACHE["nc"] = _build()
    return _CACHE["nc"]


def _in_maps(inputs):
    def f(a):
        return np.ascontiguousarray(np.asarray(a, dtype=np.float32))

    x = f(inputs["x"])
    shared = {k: f(inputs[k]) for k in ("Wg", "bg", "Wlr", "blr", "Wo", "bo")}
    return [{"x": x[i], **shared} for i in range(N)]


def _run(in_maps, **kw):
    from concourse.bass_utils import run_bass_kernel_spmd

    nc = _get_nc()
    return run_bass_kernel_spmd(nc, in_maps, list(range(N)), **kw)


def kernel(**inputs) -> np.ndarray:
    res = _run(_in_maps(inputs))
    return np.stack([res.results[i]["out"] for i in range(N)]).astype(np.float32)

